# revision 23
# baseline (speedup 1.0000x reference)
"""Trainium2 Bass kernel for DifferentiableWeightedRadialFrequencyLoss.

Math:
  loss = sum_{n,c,u,v} Wmap[u,v] * |FFT2(pred-gt)[u,v]|^2 / size
with Wmap = sum_b w_b * mask_b (bands disjoint), in unshifted (ifftshift)
frequency coordinates.

Device algorithm (per core, 12 images = 6 pairs):
  - pack two real images per complex FFT: Z = E1 + i*E2 (Wmap is symmetric
    under (u,v) -> (-u,-v), so cross terms cancel exactly).
  - FFT2 as two matmul stages with the symmetric ortho DFT matrix D:
      out1 = Z^T @ D      (stage 1)
      F^T  = D @ out1     (stage 2)
    each complex product via 3-multiplication Karatsuba:
      (A+iB)@(C+iD): m1=A@(C+D), m2=(A+B)@D, m3=(B-A)@C
                     real=m1-m2, imag=m1+m3
    with the constant-side combos (Dr+Di, Di-Dr) precomputed on host.
  - weighted power: P = Fr^2 + Fi^2 (ACT squares), acc += row-sum(P .* W^T)
    via DVE scalar_tensor_tensor accum_out.
Host: shard batch across 8 cores, sum partial accumulators, divide by size.
"""

import numpy as np
import ml_dtypes

import concourse.bass as bass
import concourse.bacc as bacc
import concourse.tile as tile
from concourse import mybir
from concourse.bass_utils import run_bass_kernel_spmd

N_CORES = 8
N, C, H = 32, 3, 512
NUM_BANDS = 16
IMGS_PER_CORE = (N // N_CORES) * C          # 12
PAIRS = IMGS_PER_CORE // 2                  # 6
F32 = mybir.dt.float32
BF16 = mybir.dt.bfloat16
ALU = mybir.AluOpType

# exposed for test.py introspection
last_results = None
last_nc = None
last_in_maps = None


def _build_nc(s2_3m=True, combo_eng="dve", xs_chunked=True,
              zbufs=3, o1bufs=2, ldbufs=2, tadd_eng="dve", spbufs=3,
              repeat=None):
    """repeat=N wraps the whole per-core body in a hardware For_i loop that
    re-executes it N times (identical work each iteration, including input
    and constant DMA).  Used by test.py to measure steady-state per-execution
    HW time by differencing two repeat counts; repeat=None is the normal
    single-shot kernel."""
    from contextlib import nullcontext
    nc = bacc.Bacc("TRN2", target_bir_lowering=False, debug=False,
                   num_devices=N_CORES)
    pred = nc.dram_tensor("pred", [IMGS_PER_CORE, H, H], F32, kind="ExternalInput")
    gt = nc.dram_tensor("gt", [IMGS_PER_CORE, H, H], F32, kind="ExternalInput")
    d_r = nc.dram_tensor("d_r", [H, H], BF16, kind="ExternalInput")   # Dr
    d_i = nc.dram_tensor("d_i", [H, H], BF16, kind="ExternalInput")   # Di
    d_p = nc.dram_tensor("d_p", [H, H], BF16, kind="ExternalInput")   # Dr+Di
    d_m = nc.dram_tensor("d_m", [H, H], BF16, kind="ExternalInput")   # Di-Dr
    d_n = nc.dram_tensor("d_n", [H, H], BF16, kind="ExternalInput")   # -Di
    wt = nc.dram_tensor("wt", [H, H], BF16, kind="ExternalInput")     # W^T
    out = nc.dram_tensor("out", [128, PAIRS], F32, kind="ExternalOutput")

    def r4(ap):  # [512, 512] dram view -> [128 part, 4 chunks, 512]
        return ap.rearrange("(c p) w -> p c w", p=128)

    with tile.TileContext(nc) as tc:
        with (
            tc.tile_pool(name="consts", bufs=1) as consts,
            tc.tile_pool(name="loads", bufs=ldbufs) as loads,
            tc.tile_pool(name="zpool", bufs=zbufs) as zpool,
            tc.tile_pool(name="o1pool", bufs=o1bufs) as o1pool,
            tc.tile_pool(name="spool", bufs=spbufs) as spool,
            tc.tile_pool(name="bigsc", bufs=2) as bigsc,
            tc.tile_pool(name="ps1a", bufs=2, space="PSUM") as ps1a,
            tc.tile_pool(name="ps1bc", bufs=1, space="PSUM") as ps1bc,
            tc.tile_pool(name="ps2a", bufs=2, space="PSUM") as ps2a,
            tc.tile_pool(name="ps2bc", bufs=1, space="PSUM") as ps2bc,
        ):
            dr_sb = consts.tile([128, 4, H], BF16)
            di_sb = consts.tile([128, 4, H], BF16)
            dp_sb = consts.tile([128, 4, H], BF16)
            dm_sb = consts.tile([128, 4, H], BF16)
            dn_sb = None if s2_3m else consts.tile([128, 4, H], BF16)
            wt_sb = consts.tile([128, 4, H], BF16)
            acc = consts.tile([128, PAIRS], F32)
            # warmup tile: keep PE busy during the initial DMA lead-in so
            # the HAM clock-gate is at full rate when real matmuls start.
            warm = consts.tile([128, H], BF16)
            nc.vector.memset(warm[:], 0.0)
            wps = ps2bc.tile([128, H], F32, tag="c2")
            NWARM = 16
            for i in range(NWARM):
                nc.tensor.matmul(wps[:], warm[:, 0:128], warm[:],
                                 start=(i == 0), stop=(i == NWARM - 1))

            rep_ctx = (
                tc.For_i(0, repeat, 1,
                         hint_engines=(mybir.EngineType.PE,
                                       mybir.EngineType.DVE))
                if repeat is not None else nullcontext()
            )
            with rep_ctx:
              for pr in range(PAIRS):
                if pr == 0:
                    pass
                i1, i2 = 2 * pr, 2 * pr + 1
                p1t = loads.tile([128, 4, H], F32, tag="p1t")
                g1t = loads.tile([128, 4, H], F32, tag="g1t")
                p2t = loads.tile([128, 4, H], F32, tag="p2t")
                g2t = loads.tile([128, 4, H], F32, tag="g2t")
                nc.sync.dma_start(out=p1t[:], in_=r4(pred.ap()[i1]))
                nc.sync.dma_start(out=g1t[:], in_=r4(gt.ap()[i1]))
                nc.sync.dma_start(out=p2t[:], in_=r4(pred.ap()[i2]))
                nc.sync.dma_start(out=g2t[:], in_=r4(gt.ap()[i2]))
                if pr == 0:
                    nc.sync.dma_start(out=dp_sb[:], in_=r4(d_p.ap()))
                    nc.sync.dma_start(out=di_sb[:], in_=r4(d_i.ap()))
                    nc.sync.dma_start(out=dr_sb[:], in_=r4(d_r.ap()))
                    nc.sync.dma_start(out=dm_sb[:], in_=r4(d_m.ap()))
                    if dn_sb is not None:
                        nc.sync.dma_start(out=dn_sb[:], in_=r4(d_n.ap()))
                    nc.sync.dma_start(out=wt_sb[:], in_=r4(wt.ap()))

                # data tiles: zr = E1, zi = E2, zs = zr+zi, zd = zi-zr
                zr = zpool.tile([128, 4, H], BF16, tag="zr")
                zi = zpool.tile([128, 4, H], BF16, tag="zi")
                zs = zpool.tile([128, 4, H], BF16, tag="zs")
                zd = zpool.tile([128, 4, H], BF16, tag="zd")
                ce = nc.gpsimd if combo_eng == "pool" else nc.vector
                nc.gpsimd.tensor_sub(zr[:], p1t[:], g1t[:])
                nc.vector.tensor_sub(zi[:], p2t[:], g2t[:])
                ce.tensor_add(zs[:], zr[:], zi[:])
                ce.tensor_sub(zd[:], zi[:], zr[:])

                # stage 1: out1 = Z^T @ D via 3M
                o1r = o1pool.tile([128, 4, H], BF16, tag="o1r")
                o1i = o1pool.tile([128, 4, H], BF16, tag="o1i")
                for m in range(4):
                    sl = slice(m * 128, (m + 1) * 128)
                    pa = ps1a.tile([128, H], F32, tag="a")
                    pb = ps1bc.tile([128, H], F32, tag="b")
                    for k in range(4):
                        nc.tensor.matmul(pa[:], zr[:, k, sl], dp_sb[:, k, :],
                                         start=(k == 0), stop=(k == 3))
                        nc.tensor.matmul(pb[:], zs[:, k, sl], di_sb[:, k, :],
                                         start=(k == 0), stop=(k == 3))
                    pa_sb = spool.tile([128, H], F32, tag="pas")
                    nc.scalar.copy(pa_sb[:], pa[:])
                    nc.vector.tensor_sub(o1r[:, m, :], pa_sb[:], pb[:])
                    pc = ps1bc.tile([128, H], F32, tag="c")
                    for k in range(4):
                        nc.tensor.matmul(pc[:], zd[:, k, sl], dr_sb[:, k, :],
                                         start=(k == 0), stop=(k == 3))
                    nc.vector.tensor_add(o1i[:, m, :], pa_sb[:], pc[:])
                xs = o1pool.tile([128, 4, H], BF16, tag="xs")
                if s2_3m:
                    if xs_chunked:
                        for m in range(4):
                            ce.tensor_add(xs[:, m, :], o1r[:, m, :], o1i[:, m, :])
                    else:
                        ce.tensor_add(xs[:], o1r[:], o1i[:])

                # stage 2: F^T = D @ out1 via 3M ; weighted power accumulate
                prt = bigsc.tile([128, 4, H], BF16, tag="prt")
                pit = bigsc.tile([128, 4, H], BF16, tag="pit")
                for v in range(4):
                    sl = slice(v * 128, (v + 1) * 128)
                    if s2_3m:
                        pa = ps2a.tile([128, H], F32, tag="a2")
                        pb = ps2bc.tile([128, H], F32, tag="b2")
                        for p in range(4):
                            nc.tensor.matmul(pa[:], dr_sb[:, p, sl], xs[:, p, :],
                                             start=(p == 0), stop=(p == 3))
                            nc.tensor.matmul(pb[:], dp_sb[:, p, sl], o1i[:, p, :],
                                             start=(p == 0), stop=(p == 3))
                        pa2_sb = spool.tile([128, H], F32, tag="pas2")
                        nc.scalar.copy(pa2_sb[:], pa[:])
                        fr = spool.tile([128, H], BF16, tag="fr")
                        nc.vector.tensor_sub(fr[:], pa2_sb[:], pb[:])
                        pc = ps2bc.tile([128, H], F32, tag="c2")
                        for p in range(4):
                            nc.tensor.matmul(pc[:], dm_sb[:, p, sl], o1r[:, p, :],
                                             start=(p == 0), stop=(p == 3))
                        fi = spool.tile([128, H], BF16, tag="fi")
                        nc.vector.tensor_add(fi[:], pa2_sb[:], pc[:])
                        nc.scalar.square(prt[:, v, :], fr[:])
                        nc.scalar.square(pit[:, v, :], fi[:])
                    else:
                        pa = ps2a.tile([128, H], F32, tag="a2")
                        pb = ps2a.tile([128, H], F32, tag="b2")
                        for p in range(4):
                            drp = dr_sb[:, p, sl]
                            nc.tensor.matmul(pa[:], drp, o1r[:, p, :],
                                             start=(p == 0), stop=False)
                            nc.tensor.matmul(pb[:], drp, o1i[:, p, :],
                                             start=(p == 0), stop=False)
                        for p in range(4):
                            nc.tensor.matmul(pa[:], dn_sb[:, p, sl], o1i[:, p, :],
                                             start=False, stop=(p == 3))
                            nc.tensor.matmul(pb[:], di_sb[:, p, sl], o1r[:, p, :],
                                             start=False, stop=(p == 3))
                        nc.scalar.square(prt[:, v, :], pa[:])
                        nc.scalar.square(pit[:, v, :], pb[:])
                t = bigsc.tile([128, 4, H], BF16, tag="t")
                te = nc.gpsimd if tadd_eng == "pool" else nc.vector
                te.tensor_add(t[:], prt[:], pit[:])
                gs = bigsc.tile([128, 4, H], BF16, tag="t")
                se = nc.gpsimd if tadd_eng == "pool" else nc.vector
                se.scalar_tensor_tensor(
                    out=gs[:], in0=t[:], scalar=0.0, in1=wt_sb[:],
                    op0=ALU.bypass, op1=ALU.mult,
                    accum_out=acc[:, pr: pr + 1])

            nc.sync.dma_start(out=out.ap(), in_=acc[:])

    nc.compile()
    return nc


def _build_nc_v2(repeat=None, zbufs=3, o1bufs=2, ldbufs=2, spbufs=3,
                 t_eng="dve", direct_combine=False, chunk_acc="last",
                 nwarm=16, defer_s2=True, xs_big=True, zi_eng="dve"):
    """3M kernel, restructured vs _build_nc:
      - stage 2 of pair pr is emitted after stage 1 of pair pr+1 (defer_s2),
        so the PE instruction stream never stalls at the stage1->stage2
        dependency (stage2(pr)'s inputs are long ready by then)
      - the final pair's power-accumulate runs per v-chunk (chunk_acc="last")
        to shrink the kernel tail after the last matmul
      - optional variants kept as parameters for A/B: direct PSUM combines,
        engine choices, buffer depths."""
    from contextlib import nullcontext
    nc = bacc.Bacc("TRN2", target_bir_lowering=False, debug=False,
                   num_devices=N_CORES)
    pred = nc.dram_tensor("pred", [IMGS_PER_CORE, H, H], F32, kind="ExternalInput")
    gt = nc.dram_tensor("gt", [IMGS_PER_CORE, H, H], F32, kind="ExternalInput")
    d_r = nc.dram_tensor("d_r", [H, H], BF16, kind="ExternalInput")   # Dr
    d_i = nc.dram_tensor("d_i", [H, H], BF16, kind="ExternalInput")   # Di
    d_p = nc.dram_tensor("d_p", [H, H], BF16, kind="ExternalInput")   # Dr+Di
    d_m = nc.dram_tensor("d_m", [H, H], BF16, kind="ExternalInput")   # Di-Dr
    d_n = nc.dram_tensor("d_n", [H, H], BF16, kind="ExternalInput")   # -Di (unused)
    wt = nc.dram_tensor("wt", [H, H], BF16, kind="ExternalInput")     # W^T
    NACC = PAIRS * 4 if chunk_acc else PAIRS
    out = nc.dram_tensor("out", [128, NACC], F32, kind="ExternalOutput")
    # chunk_acc: True = per-v-chunk accumulate everywhere, "last" = only for
    # the final pair (shrinks the kernel tail without the per-op overhead
    # elsewhere), False = one big accumulate per pair.

    def r4(ap):
        return ap.rearrange("(c p) w -> p c w", p=128)

    with tile.TileContext(nc) as tc:
        with (
            tc.tile_pool(name="consts", bufs=1) as consts,
            tc.tile_pool(name="loads", bufs=ldbufs) as loads,
            tc.tile_pool(name="zpool", bufs=zbufs) as zpool,
            tc.tile_pool(name="o1pool", bufs=o1bufs) as o1pool,
            tc.tile_pool(name="spool", bufs=spbufs) as spool,
            tc.tile_pool(name="bigsc", bufs=2) as bigsc,
            tc.tile_pool(name="ps1a", bufs=2, space="PSUM") as ps1a,
            tc.tile_pool(name="ps1bc", bufs=1, space="PSUM") as ps1bc,
            tc.tile_pool(name="ps2a", bufs=2, space="PSUM") as ps2a,
            tc.tile_pool(name="ps2bc", bufs=1, space="PSUM") as ps2bc,
        ):
            dr_sb = consts.tile([128, 4, H], BF16)
            di_sb = consts.tile([128, 4, H], BF16)
            dp_sb = consts.tile([128, 4, H], BF16)
            dm_sb = consts.tile([128, 4, H], BF16)
            wt_sb = consts.tile([128, 4, H], BF16)
            acc = consts.tile([128, NACC], F32)
            warm = consts.tile([128, H], BF16)
            nc.vector.memset(warm[:], 0.0)
            wps = ps2bc.tile([128, H], F32, tag="c2")
            for i in range(nwarm):
                nc.tensor.matmul(wps[:], warm[:, 0:128], warm[:],
                                 start=(i == 0), stop=(i == nwarm - 1))

            if chunk_acc:
                nc.vector.memset(acc[:], 0.0)
            te = nc.gpsimd if t_eng == "pool" else nc.vector

            def stage2(o1r, o1i, xs, pr, chunked):
                # stage 2: F^T = D @ out1 via 3M
                prt = bigsc.tile([128, 4, H], BF16, tag="prt")
                pit = bigsc.tile([128, 4, H], BF16, tag="pit")
                for v in range(4):
                    sl = slice(v * 128, (v + 1) * 128)
                    pa = ps2a.tile([128, H], F32, tag="a2")
                    pb = ps2bc.tile([128, H], F32, tag="b2")
                    for p in range(4):
                        nc.tensor.matmul(pa[:], dr_sb[:, p, sl], xs[:, p, :],
                                         start=(p == 0), stop=(p == 3))
                        nc.tensor.matmul(pb[:], dp_sb[:, p, sl], o1i[:, p, :],
                                         start=(p == 0), stop=(p == 3))
                    if direct_combine:
                        pa_rd = pa
                    else:
                        pa_rd = spool.tile([128, H], F32, tag="pas2")
                        nc.scalar.copy(pa_rd[:], pa[:])
                    fr = spool.tile([128, H], BF16, tag="fr")
                    nc.vector.tensor_sub(fr[:], pa_rd[:], pb[:])
                    pc = ps2bc.tile([128, H], F32, tag="c2")
                    for p in range(4):
                        nc.tensor.matmul(pc[:], dm_sb[:, p, sl], o1r[:, p, :],
                                         start=(p == 0), stop=(p == 3))
                    fi = spool.tile([128, H], BF16, tag="fi")
                    nc.vector.tensor_add(fi[:], pa_rd[:], pc[:])
                    nc.scalar.square(prt[:, v, :], fr[:])
                    nc.scalar.square(pit[:, v, :], fi[:])
                    if chunked:
                        tch = bigsc.tile([128, H], BF16, tag="t")
                        te.tensor_add(tch[:], prt[:, v, :], pit[:, v, :])
                        gch = bigsc.tile([128, H], BF16, tag="gs")
                        col = pr * 4 + v
                        nc.vector.scalar_tensor_tensor(
                            out=gch[:], in0=tch[:], scalar=0.0,
                            in1=wt_sb[:, v, :], op0=ALU.bypass, op1=ALU.mult,
                            accum_out=acc[:, col: col + 1])
                if not chunked:
                    t = bigsc.tile([128, 4, H], BF16, tag="t")
                    te.tensor_add(t[:], prt[:], pit[:])
                    gs = bigsc.tile([128, 4, H], BF16, tag="gs")
                    col = pr * 4 if chunk_acc else pr
                    nc.vector.scalar_tensor_tensor(
                        out=gs[:], in0=t[:], scalar=0.0, in1=wt_sb[:],
                        op0=ALU.bypass, op1=ALU.mult,
                        accum_out=acc[:, col: col + 1])

            rep_ctx = (
                tc.For_i(0, repeat, 1,
                         hint_engines=(mybir.EngineType.PE,
                                       mybir.EngineType.DVE))
                if repeat is not None else nullcontext()
            )
            with rep_ctx:
              pending = None
              for pr in range(PAIRS):
                i1, i2 = 2 * pr, 2 * pr + 1
                p1t = loads.tile([128, 4, H], F32, tag="p1t")
                g1t = loads.tile([128, 4, H], F32, tag="g1t")
                p2t = loads.tile([128, 4, H], F32, tag="p2t")
                g2t = loads.tile([128, 4, H], F32, tag="g2t")
                nc.sync.dma_start(out=p1t[:], in_=r4(pred.ap()[i1]))
                nc.sync.dma_start(out=g1t[:], in_=r4(gt.ap()[i1]))
                nc.sync.dma_start(out=p2t[:], in_=r4(pred.ap()[i2]))
                nc.sync.dma_start(out=g2t[:], in_=r4(gt.ap()[i2]))
                if pr == 0:
                    nc.sync.dma_start(out=dp_sb[:], in_=r4(d_p.ap()))
                    nc.sync.dma_start(out=di_sb[:], in_=r4(d_i.ap()))
                    nc.sync.dma_start(out=dr_sb[:], in_=r4(d_r.ap()))
                    nc.sync.dma_start(out=dm_sb[:], in_=r4(d_m.ap()))
                    nc.sync.dma_start(out=wt_sb[:], in_=r4(wt.ap()))

                zr = zpool.tile([128, 4, H], BF16, tag="zr")
                zi = zpool.tile([128, 4, H], BF16, tag="zi")
                zs = zpool.tile([128, 4, H], BF16, tag="zs")
                zd = zpool.tile([128, 4, H], BF16, tag="zd")
                nc.gpsimd.tensor_sub(zr[:], p1t[:], g1t[:])
                zie = nc.gpsimd if zi_eng == "pool" else nc.vector
                zie.tensor_sub(zi[:], p2t[:], g2t[:])
                nc.vector.tensor_add(zs[:], zr[:], zi[:])
                nc.vector.tensor_sub(zd[:], zi[:], zr[:])

                # stage 1: out1 = Z^T @ D via 3M; combines read PSUM directly
                o1r = o1pool.tile([128, 4, H], BF16, tag="o1r")
                o1i = o1pool.tile([128, 4, H], BF16, tag="o1i")
                xs = o1pool.tile([128, 4, H], BF16, tag="xs")
                for m in range(4):
                    sl = slice(m * 128, (m + 1) * 128)
                    pa = ps1a.tile([128, H], F32, tag="a")
                    pb = ps1bc.tile([128, H], F32, tag="b")
                    for k in range(4):
                        nc.tensor.matmul(pa[:], zr[:, k, sl], dp_sb[:, k, :],
                                         start=(k == 0), stop=(k == 3))
                        nc.tensor.matmul(pb[:], zs[:, k, sl], di_sb[:, k, :],
                                         start=(k == 0), stop=(k == 3))
                    if direct_combine:
                        pa_rd = pa
                    else:
                        pa_rd = spool.tile([128, H], F32, tag="pas")
                        nc.scalar.copy(pa_rd[:], pa[:])
                    nc.vector.tensor_sub(o1r[:, m, :], pa_rd[:], pb[:])
                    pc = ps1bc.tile([128, H], F32, tag="c")
                    for k in range(4):
                        nc.tensor.matmul(pc[:], zd[:, k, sl], dr_sb[:, k, :],
                                         start=(k == 0), stop=(k == 3))
                    nc.vector.tensor_add(o1i[:, m, :], pa_rd[:], pc[:])
                if xs_big:
                    nc.vector.tensor_add(xs[:], o1r[:], o1i[:])
                else:
                    for m in range(4):
                        nc.vector.tensor_add(xs[:, m, :], o1r[:, m, :],
                                             o1i[:, m, :])

                def _chunked(p):
                    return chunk_acc is True or (chunk_acc == "last"
                                                 and p == PAIRS - 1)

                if not defer_s2:
                    stage2(o1r, o1i, xs, pr, _chunked(pr))
                elif pending is not None:
                    stage2(*pending, _chunked(pending[3]))
                if defer_s2:
                    pending = (o1r, o1i, xs, pr)
              if defer_s2:
                  stage2(*pending, _chunked(pending[3]))

              nc.sync.dma_start(out=out.ap(), in_=acc[:])

    nc.compile()
    return nc


BUILD = _build_nc_v2


def kernel(predictions, ground_truths, band_weights, band_masks):
    global last_results, last_nc, last_in_maps
    pred = np.ascontiguousarray(np.asarray(predictions, dtype=np.float32))
    gt = np.ascontiguousarray(np.asarray(ground_truths, dtype=np.float32))
    bw = np.asarray(band_weights, dtype=np.float64)
    bm = np.asarray(band_masks, dtype=np.float64)

    # host-side prep of tiny replicated constants
    wmap = np.einsum('b,bhw->hw', bw, bm)          # shifted coords
    wu = np.fft.ifftshift(wmap)                     # unshifted coords
    bf = ml_dtypes.bfloat16
    wtb = np.ascontiguousarray(wu.T.astype(bf))
    j = np.arange(H, dtype=np.float64)
    ang = 2.0 * np.pi * np.outer(j, j) / H
    scale = 1.0 / np.sqrt(H)
    drm = np.cos(ang) * scale
    dim = -np.sin(ang) * scale
    drb = np.ascontiguousarray(drm.astype(bf))
    dib = np.ascontiguousarray(dim.astype(bf))
    dpb = np.ascontiguousarray((drm + dim).astype(bf))
    dmb = np.ascontiguousarray((dim - drm).astype(bf))
    dnb = np.ascontiguousarray((-dim).astype(bf))

    pred_r = pred.reshape(N_CORES, IMGS_PER_CORE, H, H)
    gt_r = gt.reshape(N_CORES, IMGS_PER_CORE, H, H)
    in_maps = [
        {
            "pred": np.ascontiguousarray(pred_r[c]),
            "gt": np.ascontiguousarray(gt_r[c]),
            "d_r": drb, "d_i": dib, "d_p": dpb, "d_m": dmb, "d_n": dnb,
            "wt": wtb,
        }
        for c in range(N_CORES)
    ]

    nc = BUILD()
    last_nc, last_in_maps = nc, in_maps
    res = run_bass_kernel_spmd(nc, in_maps, core_ids=list(range(N_CORES)))
    last_results = res
    total = np.float64(0.0)
    for r in res.results:
        total += r["out"].astype(np.float64).sum()
    loss = total / float(N * C * H * H)
    return np.float32(loss)



# revision 29
# speedup vs baseline: 1.0255x; 1.0255x over previous
"""Trainium2 Bass kernel for DifferentiableWeightedRadialFrequencyLoss.

Math:
  loss = sum_{n,c,u,v} Wmap[u,v] * |FFT2(pred-gt)[u,v]|^2 / size
with Wmap = sum_b w_b * mask_b (bands disjoint), in unshifted (ifftshift)
frequency coordinates.

Device algorithm (per core, 12 images = 6 pairs):
  - pack two real images per complex FFT: Z = E1 + i*E2 (Wmap is symmetric
    under (u,v) -> (-u,-v), so cross terms cancel exactly).
  - FFT2 as two matmul stages with the symmetric ortho DFT matrix D:
      out1 = Z^T @ D      (stage 1)
      F^T  = D @ out1     (stage 2)
    each complex product via 3-multiplication Karatsuba:
      (A+iB)@(C+iD): m1=A@(C+D), m2=(A+B)@D, m3=(B-A)@C
                     real=m1-m2, imag=m1+m3
    with the constant-side combos (Dr+Di, Di-Dr) precomputed on host.
  - weighted power: P = Fr^2 + Fi^2 (ACT squares), acc += row-sum(P .* W^T)
    via DVE scalar_tensor_tensor accum_out.
Host: shard batch across 8 cores, sum partial accumulators, divide by size.
"""

import numpy as np
import ml_dtypes

import concourse.bass as bass
import concourse.bacc as bacc
import concourse.tile as tile
from concourse import mybir
from concourse.bass_utils import run_bass_kernel_spmd

N_CORES = 8
N, C, H = 32, 3, 512
NUM_BANDS = 16
IMGS_PER_CORE = (N // N_CORES) * C          # 12
PAIRS = IMGS_PER_CORE // 2                  # 6
F32 = mybir.dt.float32
BF16 = mybir.dt.bfloat16
ALU = mybir.AluOpType

# exposed for test.py introspection
last_results = None
last_nc = None
last_in_maps = None


def _build_nc(s2_3m=True, combo_eng="dve", xs_chunked=True,
              zbufs=3, o1bufs=2, ldbufs=2, tadd_eng="dve", spbufs=3,
              repeat=None):
    """repeat=N wraps the whole per-core body in a hardware For_i loop that
    re-executes it N times (identical work each iteration, including input
    and constant DMA).  Used by test.py to measure steady-state per-execution
    HW time by differencing two repeat counts; repeat=None is the normal
    single-shot kernel."""
    from contextlib import nullcontext
    nc = bacc.Bacc("TRN2", target_bir_lowering=False, debug=False,
                   num_devices=N_CORES)
    pred = nc.dram_tensor("pred", [IMGS_PER_CORE, H, H], F32, kind="ExternalInput")
    gt = nc.dram_tensor("gt", [IMGS_PER_CORE, H, H], F32, kind="ExternalInput")
    d_r = nc.dram_tensor("d_r", [H, H], BF16, kind="ExternalInput")   # Dr
    d_i = nc.dram_tensor("d_i", [H, H], BF16, kind="ExternalInput")   # Di
    d_p = nc.dram_tensor("d_p", [H, H], BF16, kind="ExternalInput")   # Dr+Di
    d_m = nc.dram_tensor("d_m", [H, H], BF16, kind="ExternalInput")   # Di-Dr
    d_n = nc.dram_tensor("d_n", [H, H], BF16, kind="ExternalInput")   # -Di
    wt = nc.dram_tensor("wt", [H, H], BF16, kind="ExternalInput")     # W^T
    out = nc.dram_tensor("out", [128, PAIRS], F32, kind="ExternalOutput")

    def r4(ap):  # [512, 512] dram view -> [128 part, 4 chunks, 512]
        return ap.rearrange("(c p) w -> p c w", p=128)

    with tile.TileContext(nc) as tc:
        with (
            tc.tile_pool(name="consts", bufs=1) as consts,
            tc.tile_pool(name="loads", bufs=ldbufs) as loads,
            tc.tile_pool(name="zpool", bufs=zbufs) as zpool,
            tc.tile_pool(name="o1pool", bufs=o1bufs) as o1pool,
            tc.tile_pool(name="spool", bufs=spbufs) as spool,
            tc.tile_pool(name="bigsc", bufs=2) as bigsc,
            tc.tile_pool(name="ps1a", bufs=2, space="PSUM") as ps1a,
            tc.tile_pool(name="ps1bc", bufs=1, space="PSUM") as ps1bc,
            tc.tile_pool(name="ps2a", bufs=2, space="PSUM") as ps2a,
            tc.tile_pool(name="ps2bc", bufs=1, space="PSUM") as ps2bc,
        ):
            dr_sb = consts.tile([128, 4, H], BF16)
            di_sb = consts.tile([128, 4, H], BF16)
            dp_sb = consts.tile([128, 4, H], BF16)
            dm_sb = consts.tile([128, 4, H], BF16)
            dn_sb = None if s2_3m else consts.tile([128, 4, H], BF16)
            wt_sb = consts.tile([128, 4, H], BF16)
            acc = consts.tile([128, PAIRS], F32)
            # warmup tile: keep PE busy during the initial DMA lead-in so
            # the HAM clock-gate is at full rate when real matmuls start.
            warm = consts.tile([128, H], BF16)
            nc.vector.memset(warm[:], 0.0)
            wps = ps2bc.tile([128, H], F32, tag="c2")
            NWARM = 16
            for i in range(NWARM):
                nc.tensor.matmul(wps[:], warm[:, 0:128], warm[:],
                                 start=(i == 0), stop=(i == NWARM - 1))

            rep_ctx = (
                tc.For_i(0, repeat, 1,
                         hint_engines=(mybir.EngineType.PE,
                                       mybir.EngineType.DVE))
                if repeat is not None else nullcontext()
            )
            with rep_ctx:
              for pr in range(PAIRS):
                if pr == 0:
                    pass
                i1, i2 = 2 * pr, 2 * pr + 1
                p1t = loads.tile([128, 4, H], F32, tag="p1t")
                g1t = loads.tile([128, 4, H], F32, tag="g1t")
                p2t = loads.tile([128, 4, H], F32, tag="p2t")
                g2t = loads.tile([128, 4, H], F32, tag="g2t")
                nc.sync.dma_start(out=p1t[:], in_=r4(pred.ap()[i1]))
                nc.sync.dma_start(out=g1t[:], in_=r4(gt.ap()[i1]))
                nc.sync.dma_start(out=p2t[:], in_=r4(pred.ap()[i2]))
                nc.sync.dma_start(out=g2t[:], in_=r4(gt.ap()[i2]))
                if pr == 0:
                    nc.sync.dma_start(out=dp_sb[:], in_=r4(d_p.ap()))
                    nc.sync.dma_start(out=di_sb[:], in_=r4(d_i.ap()))
                    nc.sync.dma_start(out=dr_sb[:], in_=r4(d_r.ap()))
                    nc.sync.dma_start(out=dm_sb[:], in_=r4(d_m.ap()))
                    if dn_sb is not None:
                        nc.sync.dma_start(out=dn_sb[:], in_=r4(d_n.ap()))
                    nc.sync.dma_start(out=wt_sb[:], in_=r4(wt.ap()))

                # data tiles: zr = E1, zi = E2, zs = zr+zi, zd = zi-zr
                zr = zpool.tile([128, 4, H], BF16, tag="zr")
                zi = zpool.tile([128, 4, H], BF16, tag="zi")
                zs = zpool.tile([128, 4, H], BF16, tag="zs")
                zd = zpool.tile([128, 4, H], BF16, tag="zd")
                ce = nc.gpsimd if combo_eng == "pool" else nc.vector
                nc.gpsimd.tensor_sub(zr[:], p1t[:], g1t[:])
                nc.vector.tensor_sub(zi[:], p2t[:], g2t[:])
                ce.tensor_add(zs[:], zr[:], zi[:])
                ce.tensor_sub(zd[:], zi[:], zr[:])

                # stage 1: out1 = Z^T @ D via 3M
                o1r = o1pool.tile([128, 4, H], BF16, tag="o1r")
                o1i = o1pool.tile([128, 4, H], BF16, tag="o1i")
                for m in range(4):
                    sl = slice(m * 128, (m + 1) * 128)
                    pa = ps1a.tile([128, H], F32, tag="a")
                    pb = ps1bc.tile([128, H], F32, tag="b")
                    for k in range(4):
                        nc.tensor.matmul(pa[:], zr[:, k, sl], dp_sb[:, k, :],
                                         start=(k == 0), stop=(k == 3))
                        nc.tensor.matmul(pb[:], zs[:, k, sl], di_sb[:, k, :],
                                         start=(k == 0), stop=(k == 3))
                    pa_sb = spool.tile([128, H], F32, tag="pas")
                    nc.scalar.copy(pa_sb[:], pa[:])
                    nc.vector.tensor_sub(o1r[:, m, :], pa_sb[:], pb[:])
                    pc = ps1bc.tile([128, H], F32, tag="c")
                    for k in range(4):
                        nc.tensor.matmul(pc[:], zd[:, k, sl], dr_sb[:, k, :],
                                         start=(k == 0), stop=(k == 3))
                    nc.vector.tensor_add(o1i[:, m, :], pa_sb[:], pc[:])
                xs = o1pool.tile([128, 4, H], BF16, tag="xs")
                if s2_3m:
                    if xs_chunked:
                        for m in range(4):
                            ce.tensor_add(xs[:, m, :], o1r[:, m, :], o1i[:, m, :])
                    else:
                        ce.tensor_add(xs[:], o1r[:], o1i[:])

                # stage 2: F^T = D @ out1 via 3M ; weighted power accumulate
                prt = bigsc.tile([128, 4, H], BF16, tag="prt")
                pit = bigsc.tile([128, 4, H], BF16, tag="pit")
                for v in range(4):
                    sl = slice(v * 128, (v + 1) * 128)
                    if s2_3m:
                        pa = ps2a.tile([128, H], F32, tag="a2")
                        pb = ps2bc.tile([128, H], F32, tag="b2")
                        for p in range(4):
                            nc.tensor.matmul(pa[:], dr_sb[:, p, sl], xs[:, p, :],
                                             start=(p == 0), stop=(p == 3))
                            nc.tensor.matmul(pb[:], dp_sb[:, p, sl], o1i[:, p, :],
                                             start=(p == 0), stop=(p == 3))
                        pa2_sb = spool.tile([128, H], F32, tag="pas2")
                        nc.scalar.copy(pa2_sb[:], pa[:])
                        fr = spool.tile([128, H], BF16, tag="fr")
                        nc.vector.tensor_sub(fr[:], pa2_sb[:], pb[:])
                        pc = ps2bc.tile([128, H], F32, tag="c2")
                        for p in range(4):
                            nc.tensor.matmul(pc[:], dm_sb[:, p, sl], o1r[:, p, :],
                                             start=(p == 0), stop=(p == 3))
                        fi = spool.tile([128, H], BF16, tag="fi")
                        nc.vector.tensor_add(fi[:], pa2_sb[:], pc[:])
                        nc.scalar.square(prt[:, v, :], fr[:])
                        nc.scalar.square(pit[:, v, :], fi[:])
                    else:
                        pa = ps2a.tile([128, H], F32, tag="a2")
                        pb = ps2a.tile([128, H], F32, tag="b2")
                        for p in range(4):
                            drp = dr_sb[:, p, sl]
                            nc.tensor.matmul(pa[:], drp, o1r[:, p, :],
                                             start=(p == 0), stop=False)
                            nc.tensor.matmul(pb[:], drp, o1i[:, p, :],
                                             start=(p == 0), stop=False)
                        for p in range(4):
                            nc.tensor.matmul(pa[:], dn_sb[:, p, sl], o1i[:, p, :],
                                             start=False, stop=(p == 3))
                            nc.tensor.matmul(pb[:], di_sb[:, p, sl], o1r[:, p, :],
                                             start=False, stop=(p == 3))
                        nc.scalar.square(prt[:, v, :], pa[:])
                        nc.scalar.square(pit[:, v, :], pb[:])
                t = bigsc.tile([128, 4, H], BF16, tag="t")
                te = nc.gpsimd if tadd_eng == "pool" else nc.vector
                te.tensor_add(t[:], prt[:], pit[:])
                gs = bigsc.tile([128, 4, H], BF16, tag="t")
                se = nc.gpsimd if tadd_eng == "pool" else nc.vector
                se.scalar_tensor_tensor(
                    out=gs[:], in0=t[:], scalar=0.0, in1=wt_sb[:],
                    op0=ALU.bypass, op1=ALU.mult,
                    accum_out=acc[:, pr: pr + 1])

            nc.sync.dma_start(out=out.ap(), in_=acc[:])

    nc.compile()
    return nc


def _build_nc_v2(repeat=None, zbufs=3, o1bufs=2, ldbufs=2, spbufs=3,
                 t_eng="dve", direct_combine=False, chunk_acc="last",
                 nwarm=16, defer_s2=True, xs_big=True, zi_eng="dve",
                 group_seq=True, stag_reset=True):
    """3M kernel, restructured vs _build_nc:
      - stage 2 of pair pr is emitted after stage 1 of pair pr+1 (defer_s2),
        so the PE instruction stream never stalls at the stage1->stage2
        dependency (stage2(pr)'s inputs are long ready by then)
      - the final pair's power-accumulate runs per v-chunk (chunk_acc="last")
        to shrink the kernel tail after the last matmul
      - optional variants kept as parameters for A/B: direct PSUM combines,
        engine choices, buffer depths."""
    from contextlib import nullcontext
    nc = bacc.Bacc("TRN2", target_bir_lowering=False, debug=False,
                   num_devices=N_CORES)
    pred = nc.dram_tensor("pred", [IMGS_PER_CORE, H, H], F32, kind="ExternalInput")
    gt = nc.dram_tensor("gt", [IMGS_PER_CORE, H, H], F32, kind="ExternalInput")
    d_r = nc.dram_tensor("d_r", [H, H], BF16, kind="ExternalInput")   # Dr
    d_i = nc.dram_tensor("d_i", [H, H], BF16, kind="ExternalInput")   # Di
    d_p = nc.dram_tensor("d_p", [H, H], BF16, kind="ExternalInput")   # Dr+Di
    d_m = nc.dram_tensor("d_m", [H, H], BF16, kind="ExternalInput")   # Di-Dr
    d_n = nc.dram_tensor("d_n", [H, H], BF16, kind="ExternalInput")   # -Di (unused)
    wt = nc.dram_tensor("wt", [H, H], BF16, kind="ExternalInput")     # W^T
    NACC = PAIRS * 4 if chunk_acc else PAIRS
    out = nc.dram_tensor("out", [128, NACC], F32, kind="ExternalOutput")
    # chunk_acc: True = per-v-chunk accumulate everywhere, "last" = only for
    # the final pair (shrinks the kernel tail without the per-op overhead
    # elsewhere), False = one big accumulate per pair.

    def r4(ap):
        return ap.rearrange("(c p) w -> p c w", p=128)

    with tile.TileContext(nc) as tc:
        with (
            tc.tile_pool(name="consts", bufs=1) as consts,
            tc.tile_pool(name="loads", bufs=ldbufs) as loads,
            tc.tile_pool(name="zpool", bufs=zbufs) as zpool,
            tc.tile_pool(name="o1pool", bufs=o1bufs) as o1pool,
            tc.tile_pool(name="spool", bufs=spbufs) as spool,
            tc.tile_pool(name="bigsc", bufs=2) as bigsc,
            tc.tile_pool(name="ps1a", bufs=2, space="PSUM") as ps1a,
            tc.tile_pool(name="ps1bc", bufs=1, space="PSUM") as ps1bc,
            tc.tile_pool(name="ps2a", bufs=2, space="PSUM") as ps2a,
            tc.tile_pool(name="ps2bc", bufs=1, space="PSUM") as ps2bc,
        ):
            dr_sb = consts.tile([128, 4, H], BF16)
            di_sb = consts.tile([128, 4, H], BF16)
            dp_sb = consts.tile([128, 4, H], BF16)
            dm_sb = consts.tile([128, 4, H], BF16)
            wt_sb = consts.tile([128, 4, H], BF16)
            acc = consts.tile([128, NACC], F32)
            warm = consts.tile([128, H], BF16)
            nc.vector.memset(warm[:], 0.0)
            wps = ps2bc.tile([128, H], F32, tag="c2")
            for i in range(nwarm):
                nc.tensor.matmul(wps[:], warm[:, 0:128], warm[:],
                                 start=(i == 0), stop=(i == nwarm - 1))

            if chunk_acc:
                nc.vector.memset(acc[:], 0.0)
            te = nc.gpsimd if t_eng == "pool" else nc.vector

            def stage2(o1r, o1i, xs, pr, chunked):
                # stage 2: F^T = D @ out1 via 3M
                prt = bigsc.tile([128, 4, H], BF16, tag="prt")
                pit = bigsc.tile([128, 4, H], BF16, tag="pit")
                for v in range(4):
                    sl = slice(v * 128, (v + 1) * 128)
                    pa = ps2a.tile([128, H], F32, tag="a2")
                    pb = ps2bc.tile([128, H], F32, tag="b2")
                    if group_seq:
                        for p in range(4):
                            nc.tensor.matmul(pa[:], dr_sb[:, p, sl],
                                             xs[:, p, :],
                                             start=(p == 0), stop=(p == 3))
                        for p in range(4):
                            nc.tensor.matmul(pb[:], dp_sb[:, p, sl],
                                             o1i[:, p, :],
                                             start=(p == 0), stop=(p == 3))
                    else:
                        for p in range(4):
                            nc.tensor.matmul(pa[:], dr_sb[:, p, sl],
                                             xs[:, p, :],
                                             start=(p == 0), stop=(p == 3))
                            nc.tensor.matmul(pb[:], dp_sb[:, p, sl],
                                             o1i[:, p, :],
                                             start=(p == 0), stop=(p == 3))
                    if direct_combine:
                        pa_rd = pa
                    else:
                        pa_rd = spool.tile([128, H], F32, tag="pas2")
                        nc.scalar.copy(pa_rd[:], pa[:])
                    fr = spool.tile([128, H], BF16, tag="fr")
                    nc.vector.tensor_sub(fr[:], pa_rd[:], pb[:])
                    pc = ps2bc.tile([128, H], F32, tag="c2")
                    for p in range(4):
                        nc.tensor.matmul(pc[:], dm_sb[:, p, sl], o1r[:, p, :],
                                         start=(p == 0), stop=(p == 3))
                    fi = spool.tile([128, H], BF16, tag="fi")
                    nc.vector.tensor_add(fi[:], pa_rd[:], pc[:])
                    nc.scalar.square(prt[:, v, :], fr[:])
                    nc.scalar.square(pit[:, v, :], fi[:])
                    if chunked:
                        tch = bigsc.tile([128, H], BF16, tag="t")
                        te.tensor_add(tch[:], prt[:, v, :], pit[:, v, :])
                        gch = bigsc.tile([128, H], BF16, tag="gs")
                        col = pr * 4 + v
                        nc.vector.scalar_tensor_tensor(
                            out=gch[:], in0=tch[:], scalar=0.0,
                            in1=wt_sb[:, v, :], op0=ALU.bypass, op1=ALU.mult,
                            accum_out=acc[:, col: col + 1])
                if not chunked:
                    t = bigsc.tile([128, 4, H], BF16, tag="t")
                    te.tensor_add(t[:], prt[:], pit[:])
                    gs = bigsc.tile([128, 4, H], BF16, tag="gs")
                    col = pr * 4 if chunk_acc else pr
                    nc.vector.scalar_tensor_tensor(
                        out=gs[:], in0=t[:], scalar=0.0, in1=wt_sb[:],
                        op0=ALU.bypass, op1=ALU.mult,
                        accum_out=acc[:, col: col + 1])

            rep_ctx = (
                tc.For_i(0, repeat, 1,
                         hint_engines=(mybir.EngineType.PE,
                                       mybir.EngineType.DVE),
                         staggered_reset=stag_reset)
                if repeat is not None else nullcontext()
            )
            with rep_ctx:
              pending = None
              for pr in range(PAIRS):
                i1, i2 = 2 * pr, 2 * pr + 1
                p1t = loads.tile([128, 4, H], F32, tag="p1t")
                g1t = loads.tile([128, 4, H], F32, tag="g1t")
                p2t = loads.tile([128, 4, H], F32, tag="p2t")
                g2t = loads.tile([128, 4, H], F32, tag="g2t")
                nc.sync.dma_start(out=p1t[:], in_=r4(pred.ap()[i1]))
                nc.sync.dma_start(out=g1t[:], in_=r4(gt.ap()[i1]))
                nc.sync.dma_start(out=p2t[:], in_=r4(pred.ap()[i2]))
                nc.sync.dma_start(out=g2t[:], in_=r4(gt.ap()[i2]))
                if pr == 0:
                    nc.sync.dma_start(out=dp_sb[:], in_=r4(d_p.ap()))
                    nc.sync.dma_start(out=di_sb[:], in_=r4(d_i.ap()))
                    nc.sync.dma_start(out=dr_sb[:], in_=r4(d_r.ap()))
                    nc.sync.dma_start(out=dm_sb[:], in_=r4(d_m.ap()))
                    nc.sync.dma_start(out=wt_sb[:], in_=r4(wt.ap()))

                zr = zpool.tile([128, 4, H], BF16, tag="zr")
                zi = zpool.tile([128, 4, H], BF16, tag="zi")
                zs = zpool.tile([128, 4, H], BF16, tag="zs")
                zd = zpool.tile([128, 4, H], BF16, tag="zd")
                nc.gpsimd.tensor_sub(zr[:], p1t[:], g1t[:])
                zie = nc.gpsimd if zi_eng == "pool" else nc.vector
                zie.tensor_sub(zi[:], p2t[:], g2t[:])
                nc.vector.tensor_add(zs[:], zr[:], zi[:])
                nc.vector.tensor_sub(zd[:], zi[:], zr[:])

                # stage 1: out1 = Z^T @ D via 3M; combines read PSUM directly
                o1r = o1pool.tile([128, 4, H], BF16, tag="o1r")
                o1i = o1pool.tile([128, 4, H], BF16, tag="o1i")
                xs = o1pool.tile([128, 4, H], BF16, tag="xs")
                for m in range(4):
                    sl = slice(m * 128, (m + 1) * 128)
                    pa = ps1a.tile([128, H], F32, tag="a")
                    pb = ps1bc.tile([128, H], F32, tag="b")
                    if group_seq:
                        for k in range(4):
                            nc.tensor.matmul(pa[:], zr[:, k, sl],
                                             dp_sb[:, k, :],
                                             start=(k == 0), stop=(k == 3))
                        for k in range(4):
                            nc.tensor.matmul(pb[:], zs[:, k, sl],
                                             di_sb[:, k, :],
                                             start=(k == 0), stop=(k == 3))
                    else:
                        for k in range(4):
                            nc.tensor.matmul(pa[:], zr[:, k, sl],
                                             dp_sb[:, k, :],
                                             start=(k == 0), stop=(k == 3))
                            nc.tensor.matmul(pb[:], zs[:, k, sl],
                                             di_sb[:, k, :],
                                             start=(k == 0), stop=(k == 3))
                    if direct_combine:
                        pa_rd = pa
                    else:
                        pa_rd = spool.tile([128, H], F32, tag="pas")
                        nc.scalar.copy(pa_rd[:], pa[:])
                    nc.vector.tensor_sub(o1r[:, m, :], pa_rd[:], pb[:])
                    pc = ps1bc.tile([128, H], F32, tag="c")
                    for k in range(4):
                        nc.tensor.matmul(pc[:], zd[:, k, sl], dr_sb[:, k, :],
                                         start=(k == 0), stop=(k == 3))
                    nc.vector.tensor_add(o1i[:, m, :], pa_rd[:], pc[:])
                if xs_big:
                    nc.vector.tensor_add(xs[:], o1r[:], o1i[:])
                else:
                    for m in range(4):
                        nc.vector.tensor_add(xs[:, m, :], o1r[:, m, :],
                                             o1i[:, m, :])

                def _chunked(p):
                    return chunk_acc is True or (chunk_acc == "last"
                                                 and p == PAIRS - 1)

                if not defer_s2:
                    stage2(o1r, o1i, xs, pr, _chunked(pr))
                elif pending is not None:
                    stage2(*pending, _chunked(pending[3]))
                if defer_s2:
                    pending = (o1r, o1i, xs, pr)
              if defer_s2:
                  stage2(*pending, _chunked(pending[3]))

              nc.sync.dma_start(out=out.ap(), in_=acc[:])

    nc.compile()
    return nc


BUILD = _build_nc_v2


def kernel(predictions, ground_truths, band_weights, band_masks):
    global last_results, last_nc, last_in_maps
    pred = np.ascontiguousarray(np.asarray(predictions, dtype=np.float32))
    gt = np.ascontiguousarray(np.asarray(ground_truths, dtype=np.float32))
    bw = np.asarray(band_weights, dtype=np.float64)
    bm = np.asarray(band_masks, dtype=np.float64)

    # host-side prep of tiny replicated constants
    wmap = np.einsum('b,bhw->hw', bw, bm)          # shifted coords
    wu = np.fft.ifftshift(wmap)                     # unshifted coords
    bf = ml_dtypes.bfloat16
    wtb = np.ascontiguousarray(wu.T.astype(bf))
    j = np.arange(H, dtype=np.float64)
    ang = 2.0 * np.pi * np.outer(j, j) / H
    scale = 1.0 / np.sqrt(H)
    drm = np.cos(ang) * scale
    dim = -np.sin(ang) * scale
    drb = np.ascontiguousarray(drm.astype(bf))
    dib = np.ascontiguousarray(dim.astype(bf))
    dpb = np.ascontiguousarray((drm + dim).astype(bf))
    dmb = np.ascontiguousarray((dim - drm).astype(bf))
    dnb = np.ascontiguousarray((-dim).astype(bf))

    pred_r = pred.reshape(N_CORES, IMGS_PER_CORE, H, H)
    gt_r = gt.reshape(N_CORES, IMGS_PER_CORE, H, H)
    in_maps = [
        {
            "pred": np.ascontiguousarray(pred_r[c]),
            "gt": np.ascontiguousarray(gt_r[c]),
            "d_r": drb, "d_i": dib, "d_p": dpb, "d_m": dmb, "d_n": dnb,
            "wt": wtb,
        }
        for c in range(N_CORES)
    ]

    nc = BUILD()
    last_nc, last_in_maps = nc, in_maps
    res = run_bass_kernel_spmd(nc, in_maps, core_ids=list(range(N_CORES)))
    last_results = res
    total = np.float64(0.0)
    for r in res.results:
        total += r["out"].astype(np.float64).sum()
    loss = total / float(N * C * H * H)
    return np.float32(loss)



# revision 30
# speedup vs baseline: 1.0296x; 1.0040x over previous
"""Trainium2 Bass kernel for DifferentiableWeightedRadialFrequencyLoss.

Math:
  loss = sum_{n,c,u,v} Wmap[u,v] * |FFT2(pred-gt)[u,v]|^2 / size
with Wmap = sum_b w_b * mask_b (bands disjoint), in unshifted (ifftshift)
frequency coordinates.

Device algorithm (per core, 12 images = 6 pairs):
  - pack two real images per complex FFT: Z = E1 + i*E2 (Wmap is symmetric
    under (u,v) -> (-u,-v), so cross terms cancel exactly).
  - FFT2 as two matmul stages with the symmetric ortho DFT matrix D:
      out1 = Z^T @ D      (stage 1)
      F^T  = D @ out1     (stage 2)
    each complex product via 3-multiplication Karatsuba:
      (A+iB)@(C+iD): m1=A@(C+D), m2=(A+B)@D, m3=(B-A)@C
                     real=m1-m2, imag=m1+m3
    with the constant-side combos (Dr+Di, Di-Dr) precomputed on host.
  - weighted power: P = Fr^2 + Fi^2 (ACT squares), acc += row-sum(P .* W^T)
    via DVE scalar_tensor_tensor accum_out.
Host: shard batch across 8 cores, sum partial accumulators, divide by size.
"""

import numpy as np
import ml_dtypes

import concourse.bass as bass
import concourse.bacc as bacc
import concourse.tile as tile
from concourse import mybir
from concourse.bass_utils import run_bass_kernel_spmd

N_CORES = 8
N, C, H = 32, 3, 512
NUM_BANDS = 16
IMGS_PER_CORE = (N // N_CORES) * C          # 12
PAIRS = IMGS_PER_CORE // 2                  # 6
F32 = mybir.dt.float32
BF16 = mybir.dt.bfloat16
ALU = mybir.AluOpType

# exposed for test.py introspection
last_results = None
last_nc = None
last_in_maps = None


def _build_nc(s2_3m=True, combo_eng="dve", xs_chunked=True,
              zbufs=3, o1bufs=2, ldbufs=2, tadd_eng="dve", spbufs=3,
              repeat=None):
    """repeat=N wraps the whole per-core body in a hardware For_i loop that
    re-executes it N times (identical work each iteration, including input
    and constant DMA).  Used by test.py to measure steady-state per-execution
    HW time by differencing two repeat counts; repeat=None is the normal
    single-shot kernel."""
    from contextlib import nullcontext
    nc = bacc.Bacc("TRN2", target_bir_lowering=False, debug=False,
                   num_devices=N_CORES)
    pred = nc.dram_tensor("pred", [IMGS_PER_CORE, H, H], F32, kind="ExternalInput")
    gt = nc.dram_tensor("gt", [IMGS_PER_CORE, H, H], F32, kind="ExternalInput")
    d_r = nc.dram_tensor("d_r", [H, H], BF16, kind="ExternalInput")   # Dr
    d_i = nc.dram_tensor("d_i", [H, H], BF16, kind="ExternalInput")   # Di
    d_p = nc.dram_tensor("d_p", [H, H], BF16, kind="ExternalInput")   # Dr+Di
    d_m = nc.dram_tensor("d_m", [H, H], BF16, kind="ExternalInput")   # Di-Dr
    d_n = nc.dram_tensor("d_n", [H, H], BF16, kind="ExternalInput")   # -Di
    wt = nc.dram_tensor("wt", [H, H], BF16, kind="ExternalInput")     # W^T
    out = nc.dram_tensor("out", [128, PAIRS], F32, kind="ExternalOutput")

    def r4(ap):  # [512, 512] dram view -> [128 part, 4 chunks, 512]
        return ap.rearrange("(c p) w -> p c w", p=128)

    with tile.TileContext(nc) as tc:
        with (
            tc.tile_pool(name="consts", bufs=1) as consts,
            tc.tile_pool(name="loads", bufs=ldbufs) as loads,
            tc.tile_pool(name="zpool", bufs=zbufs) as zpool,
            tc.tile_pool(name="o1pool", bufs=o1bufs) as o1pool,
            tc.tile_pool(name="spool", bufs=spbufs) as spool,
            tc.tile_pool(name="bigsc", bufs=2) as bigsc,
            tc.tile_pool(name="ps1a", bufs=2, space="PSUM") as ps1a,
            tc.tile_pool(name="ps1bc", bufs=1, space="PSUM") as ps1bc,
            tc.tile_pool(name="ps2a", bufs=2, space="PSUM") as ps2a,
            tc.tile_pool(name="ps2bc", bufs=1, space="PSUM") as ps2bc,
        ):
            dr_sb = consts.tile([128, 4, H], BF16)
            di_sb = consts.tile([128, 4, H], BF16)
            dp_sb = consts.tile([128, 4, H], BF16)
            dm_sb = consts.tile([128, 4, H], BF16)
            dn_sb = None if s2_3m else consts.tile([128, 4, H], BF16)
            wt_sb = consts.tile([128, 4, H], BF16)
            acc = consts.tile([128, PAIRS], F32)
            # warmup tile: keep PE busy during the initial DMA lead-in so
            # the HAM clock-gate is at full rate when real matmuls start.
            warm = consts.tile([128, H], BF16)
            nc.vector.memset(warm[:], 0.0)
            wps = ps2bc.tile([128, H], F32, tag="c2")
            NWARM = 16
            for i in range(NWARM):
                nc.tensor.matmul(wps[:], warm[:, 0:128], warm[:],
                                 start=(i == 0), stop=(i == NWARM - 1))

            rep_ctx = (
                tc.For_i(0, repeat, 1,
                         hint_engines=(mybir.EngineType.PE,
                                       mybir.EngineType.DVE))
                if repeat is not None else nullcontext()
            )
            with rep_ctx:
              for pr in range(PAIRS):
                if pr == 0:
                    pass
                i1, i2 = 2 * pr, 2 * pr + 1
                p1t = loads.tile([128, 4, H], F32, tag="p1t")
                g1t = loads.tile([128, 4, H], F32, tag="g1t")
                p2t = loads.tile([128, 4, H], F32, tag="p2t")
                g2t = loads.tile([128, 4, H], F32, tag="g2t")
                nc.sync.dma_start(out=p1t[:], in_=r4(pred.ap()[i1]))
                nc.sync.dma_start(out=g1t[:], in_=r4(gt.ap()[i1]))
                nc.sync.dma_start(out=p2t[:], in_=r4(pred.ap()[i2]))
                nc.sync.dma_start(out=g2t[:], in_=r4(gt.ap()[i2]))
                if pr == 0:
                    nc.sync.dma_start(out=dp_sb[:], in_=r4(d_p.ap()))
                    nc.sync.dma_start(out=di_sb[:], in_=r4(d_i.ap()))
                    nc.sync.dma_start(out=dr_sb[:], in_=r4(d_r.ap()))
                    nc.sync.dma_start(out=dm_sb[:], in_=r4(d_m.ap()))
                    if dn_sb is not None:
                        nc.sync.dma_start(out=dn_sb[:], in_=r4(d_n.ap()))
                    nc.sync.dma_start(out=wt_sb[:], in_=r4(wt.ap()))

                # data tiles: zr = E1, zi = E2, zs = zr+zi, zd = zi-zr
                zr = zpool.tile([128, 4, H], BF16, tag="zr")
                zi = zpool.tile([128, 4, H], BF16, tag="zi")
                zs = zpool.tile([128, 4, H], BF16, tag="zs")
                zd = zpool.tile([128, 4, H], BF16, tag="zd")
                ce = nc.gpsimd if combo_eng == "pool" else nc.vector
                nc.gpsimd.tensor_sub(zr[:], p1t[:], g1t[:])
                nc.vector.tensor_sub(zi[:], p2t[:], g2t[:])
                ce.tensor_add(zs[:], zr[:], zi[:])
                ce.tensor_sub(zd[:], zi[:], zr[:])

                # stage 1: out1 = Z^T @ D via 3M
                o1r = o1pool.tile([128, 4, H], BF16, tag="o1r")
                o1i = o1pool.tile([128, 4, H], BF16, tag="o1i")
                for m in range(4):
                    sl = slice(m * 128, (m + 1) * 128)
                    pa = ps1a.tile([128, H], F32, tag="a")
                    pb = ps1bc.tile([128, H], F32, tag="b")
                    for k in range(4):
                        nc.tensor.matmul(pa[:], zr[:, k, sl], dp_sb[:, k, :],
                                         start=(k == 0), stop=(k == 3))
                        nc.tensor.matmul(pb[:], zs[:, k, sl], di_sb[:, k, :],
                                         start=(k == 0), stop=(k == 3))
                    pa_sb = spool.tile([128, H], F32, tag="pas")
                    nc.scalar.copy(pa_sb[:], pa[:])
                    nc.vector.tensor_sub(o1r[:, m, :], pa_sb[:], pb[:])
                    pc = ps1bc.tile([128, H], F32, tag="c")
                    for k in range(4):
                        nc.tensor.matmul(pc[:], zd[:, k, sl], dr_sb[:, k, :],
                                         start=(k == 0), stop=(k == 3))
                    nc.vector.tensor_add(o1i[:, m, :], pa_sb[:], pc[:])
                xs = o1pool.tile([128, 4, H], BF16, tag="xs")
                if s2_3m:
                    if xs_chunked:
                        for m in range(4):
                            ce.tensor_add(xs[:, m, :], o1r[:, m, :], o1i[:, m, :])
                    else:
                        ce.tensor_add(xs[:], o1r[:], o1i[:])

                # stage 2: F^T = D @ out1 via 3M ; weighted power accumulate
                prt = bigsc.tile([128, 4, H], BF16, tag="prt")
                pit = bigsc.tile([128, 4, H], BF16, tag="pit")
                for v in range(4):
                    sl = slice(v * 128, (v + 1) * 128)
                    if s2_3m:
                        pa = ps2a.tile([128, H], F32, tag="a2")
                        pb = ps2bc.tile([128, H], F32, tag="b2")
                        for p in range(4):
                            nc.tensor.matmul(pa[:], dr_sb[:, p, sl], xs[:, p, :],
                                             start=(p == 0), stop=(p == 3))
                            nc.tensor.matmul(pb[:], dp_sb[:, p, sl], o1i[:, p, :],
                                             start=(p == 0), stop=(p == 3))
                        pa2_sb = spool.tile([128, H], F32, tag="pas2")
                        nc.scalar.copy(pa2_sb[:], pa[:])
                        fr = spool.tile([128, H], BF16, tag="fr")
                        nc.vector.tensor_sub(fr[:], pa2_sb[:], pb[:])
                        pc = ps2bc.tile([128, H], F32, tag="c2")
                        for p in range(4):
                            nc.tensor.matmul(pc[:], dm_sb[:, p, sl], o1r[:, p, :],
                                             start=(p == 0), stop=(p == 3))
                        fi = spool.tile([128, H], BF16, tag="fi")
                        nc.vector.tensor_add(fi[:], pa2_sb[:], pc[:])
                        nc.scalar.square(prt[:, v, :], fr[:])
                        nc.scalar.square(pit[:, v, :], fi[:])
                    else:
                        pa = ps2a.tile([128, H], F32, tag="a2")
                        pb = ps2a.tile([128, H], F32, tag="b2")
                        for p in range(4):
                            drp = dr_sb[:, p, sl]
                            nc.tensor.matmul(pa[:], drp, o1r[:, p, :],
                                             start=(p == 0), stop=False)
                            nc.tensor.matmul(pb[:], drp, o1i[:, p, :],
                                             start=(p == 0), stop=False)
                        for p in range(4):
                            nc.tensor.matmul(pa[:], dn_sb[:, p, sl], o1i[:, p, :],
                                             start=False, stop=(p == 3))
                            nc.tensor.matmul(pb[:], di_sb[:, p, sl], o1r[:, p, :],
                                             start=False, stop=(p == 3))
                        nc.scalar.square(prt[:, v, :], pa[:])
                        nc.scalar.square(pit[:, v, :], pb[:])
                t = bigsc.tile([128, 4, H], BF16, tag="t")
                te = nc.gpsimd if tadd_eng == "pool" else nc.vector
                te.tensor_add(t[:], prt[:], pit[:])
                gs = bigsc.tile([128, 4, H], BF16, tag="t")
                se = nc.gpsimd if tadd_eng == "pool" else nc.vector
                se.scalar_tensor_tensor(
                    out=gs[:], in0=t[:], scalar=0.0, in1=wt_sb[:],
                    op0=ALU.bypass, op1=ALU.mult,
                    accum_out=acc[:, pr: pr + 1])

            nc.sync.dma_start(out=out.ap(), in_=acc[:])

    nc.compile()
    return nc


def _build_nc_v2(repeat=None, zbufs=3, o1bufs=2, ldbufs=2, spbufs=3,
                 t_eng="dve", direct_combine=False, chunk_acc="last",
                 nwarm=16, defer_s2=True, xs_big=True, zi_eng="dve",
                 group_seq=True, stag_reset=False):
    """3M kernel, restructured vs _build_nc:
      - stage 2 of pair pr is emitted after stage 1 of pair pr+1 (defer_s2),
        so the PE instruction stream never stalls at the stage1->stage2
        dependency (stage2(pr)'s inputs are long ready by then)
      - the final pair's power-accumulate runs per v-chunk (chunk_acc="last")
        to shrink the kernel tail after the last matmul
      - optional variants kept as parameters for A/B: direct PSUM combines,
        engine choices, buffer depths."""
    from contextlib import nullcontext
    nc = bacc.Bacc("TRN2", target_bir_lowering=False, debug=False,
                   num_devices=N_CORES)
    pred = nc.dram_tensor("pred", [IMGS_PER_CORE, H, H], F32, kind="ExternalInput")
    gt = nc.dram_tensor("gt", [IMGS_PER_CORE, H, H], F32, kind="ExternalInput")
    d_r = nc.dram_tensor("d_r", [H, H], BF16, kind="ExternalInput")   # Dr
    d_i = nc.dram_tensor("d_i", [H, H], BF16, kind="ExternalInput")   # Di
    d_p = nc.dram_tensor("d_p", [H, H], BF16, kind="ExternalInput")   # Dr+Di
    d_m = nc.dram_tensor("d_m", [H, H], BF16, kind="ExternalInput")   # Di-Dr
    d_n = nc.dram_tensor("d_n", [H, H], BF16, kind="ExternalInput")   # -Di (unused)
    wt = nc.dram_tensor("wt", [H, H], BF16, kind="ExternalInput")     # W^T
    NACC = PAIRS * 4 if chunk_acc else PAIRS
    out = nc.dram_tensor("out", [128, NACC], F32, kind="ExternalOutput")
    # chunk_acc: True = per-v-chunk accumulate everywhere, "last" = only for
    # the final pair (shrinks the kernel tail without the per-op overhead
    # elsewhere), False = one big accumulate per pair.

    def r4(ap):
        return ap.rearrange("(c p) w -> p c w", p=128)

    with tile.TileContext(nc) as tc:
        with (
            tc.tile_pool(name="consts", bufs=1) as consts,
            tc.tile_pool(name="loads", bufs=ldbufs) as loads,
            tc.tile_pool(name="zpool", bufs=zbufs) as zpool,
            tc.tile_pool(name="o1pool", bufs=o1bufs) as o1pool,
            tc.tile_pool(name="spool", bufs=spbufs) as spool,
            tc.tile_pool(name="bigsc", bufs=2) as bigsc,
            tc.tile_pool(name="ps1a", bufs=2, space="PSUM") as ps1a,
            tc.tile_pool(name="ps1bc", bufs=1, space="PSUM") as ps1bc,
            tc.tile_pool(name="ps2a", bufs=2, space="PSUM") as ps2a,
            tc.tile_pool(name="ps2bc", bufs=1, space="PSUM") as ps2bc,
        ):
            dr_sb = consts.tile([128, 4, H], BF16)
            di_sb = consts.tile([128, 4, H], BF16)
            dp_sb = consts.tile([128, 4, H], BF16)
            dm_sb = consts.tile([128, 4, H], BF16)
            wt_sb = consts.tile([128, 4, H], BF16)
            acc = consts.tile([128, NACC], F32)
            warm = consts.tile([128, H], BF16)
            nc.vector.memset(warm[:], 0.0)
            wps = ps2bc.tile([128, H], F32, tag="c2")
            for i in range(nwarm):
                nc.tensor.matmul(wps[:], warm[:, 0:128], warm[:],
                                 start=(i == 0), stop=(i == nwarm - 1))

            if chunk_acc:
                nc.vector.memset(acc[:], 0.0)
            te = nc.gpsimd if t_eng == "pool" else nc.vector

            def stage2(o1r, o1i, xs, pr, chunked):
                # stage 2: F^T = D @ out1 via 3M
                prt = bigsc.tile([128, 4, H], BF16, tag="prt")
                pit = bigsc.tile([128, 4, H], BF16, tag="pit")
                for v in range(4):
                    sl = slice(v * 128, (v + 1) * 128)
                    pa = ps2a.tile([128, H], F32, tag="a2")
                    pb = ps2bc.tile([128, H], F32, tag="b2")
                    if group_seq:
                        for p in range(4):
                            nc.tensor.matmul(pa[:], dr_sb[:, p, sl],
                                             xs[:, p, :],
                                             start=(p == 0), stop=(p == 3))
                        for p in range(4):
                            nc.tensor.matmul(pb[:], dp_sb[:, p, sl],
                                             o1i[:, p, :],
                                             start=(p == 0), stop=(p == 3))
                    else:
                        for p in range(4):
                            nc.tensor.matmul(pa[:], dr_sb[:, p, sl],
                                             xs[:, p, :],
                                             start=(p == 0), stop=(p == 3))
                            nc.tensor.matmul(pb[:], dp_sb[:, p, sl],
                                             o1i[:, p, :],
                                             start=(p == 0), stop=(p == 3))
                    if direct_combine:
                        pa_rd = pa
                    else:
                        pa_rd = spool.tile([128, H], F32, tag="pas2")
                        nc.scalar.copy(pa_rd[:], pa[:])
                    fr = spool.tile([128, H], BF16, tag="fr")
                    nc.vector.tensor_sub(fr[:], pa_rd[:], pb[:])
                    pc = ps2bc.tile([128, H], F32, tag="c2")
                    for p in range(4):
                        nc.tensor.matmul(pc[:], dm_sb[:, p, sl], o1r[:, p, :],
                                         start=(p == 0), stop=(p == 3))
                    fi = spool.tile([128, H], BF16, tag="fi")
                    nc.vector.tensor_add(fi[:], pa_rd[:], pc[:])
                    nc.scalar.square(prt[:, v, :], fr[:])
                    nc.scalar.square(pit[:, v, :], fi[:])
                    if chunked:
                        tch = bigsc.tile([128, H], BF16, tag="t")
                        te.tensor_add(tch[:], prt[:, v, :], pit[:, v, :])
                        gch = bigsc.tile([128, H], BF16, tag="gs")
                        col = pr * 4 + v
                        nc.vector.scalar_tensor_tensor(
                            out=gch[:], in0=tch[:], scalar=0.0,
                            in1=wt_sb[:, v, :], op0=ALU.bypass, op1=ALU.mult,
                            accum_out=acc[:, col: col + 1])
                if not chunked:
                    t = bigsc.tile([128, 4, H], BF16, tag="t")
                    te.tensor_add(t[:], prt[:], pit[:])
                    gs = bigsc.tile([128, 4, H], BF16, tag="gs")
                    col = pr * 4 if chunk_acc else pr
                    nc.vector.scalar_tensor_tensor(
                        out=gs[:], in0=t[:], scalar=0.0, in1=wt_sb[:],
                        op0=ALU.bypass, op1=ALU.mult,
                        accum_out=acc[:, col: col + 1])

            rep_ctx = (
                tc.For_i(0, repeat, 1,
                         hint_engines=(mybir.EngineType.PE,
                                       mybir.EngineType.DVE),
                         staggered_reset=stag_reset)
                if repeat is not None else nullcontext()
            )
            with rep_ctx:
              pending = None
              for pr in range(PAIRS):
                i1, i2 = 2 * pr, 2 * pr + 1
                p1t = loads.tile([128, 4, H], F32, tag="p1t")
                g1t = loads.tile([128, 4, H], F32, tag="g1t")
                p2t = loads.tile([128, 4, H], F32, tag="p2t")
                g2t = loads.tile([128, 4, H], F32, tag="g2t")
                nc.sync.dma_start(out=p1t[:], in_=r4(pred.ap()[i1]))
                nc.sync.dma_start(out=g1t[:], in_=r4(gt.ap()[i1]))
                nc.sync.dma_start(out=p2t[:], in_=r4(pred.ap()[i2]))
                nc.sync.dma_start(out=g2t[:], in_=r4(gt.ap()[i2]))
                if pr == 0:
                    nc.sync.dma_start(out=dp_sb[:], in_=r4(d_p.ap()))
                    nc.sync.dma_start(out=di_sb[:], in_=r4(d_i.ap()))
                    nc.sync.dma_start(out=dr_sb[:], in_=r4(d_r.ap()))
                    nc.sync.dma_start(out=dm_sb[:], in_=r4(d_m.ap()))
                    nc.sync.dma_start(out=wt_sb[:], in_=r4(wt.ap()))

                zr = zpool.tile([128, 4, H], BF16, tag="zr")
                zi = zpool.tile([128, 4, H], BF16, tag="zi")
                zs = zpool.tile([128, 4, H], BF16, tag="zs")
                zd = zpool.tile([128, 4, H], BF16, tag="zd")
                nc.gpsimd.tensor_sub(zr[:], p1t[:], g1t[:])
                zie = nc.gpsimd if zi_eng == "pool" else nc.vector
                zie.tensor_sub(zi[:], p2t[:], g2t[:])
                nc.vector.tensor_add(zs[:], zr[:], zi[:])
                nc.vector.tensor_sub(zd[:], zi[:], zr[:])

                # stage 1: out1 = Z^T @ D via 3M; combines read PSUM directly
                o1r = o1pool.tile([128, 4, H], BF16, tag="o1r")
                o1i = o1pool.tile([128, 4, H], BF16, tag="o1i")
                xs = o1pool.tile([128, 4, H], BF16, tag="xs")
                for m in range(4):
                    sl = slice(m * 128, (m + 1) * 128)
                    pa = ps1a.tile([128, H], F32, tag="a")
                    pb = ps1bc.tile([128, H], F32, tag="b")
                    if group_seq:
                        for k in range(4):
                            nc.tensor.matmul(pa[:], zr[:, k, sl],
                                             dp_sb[:, k, :],
                                             start=(k == 0), stop=(k == 3))
                        for k in range(4):
                            nc.tensor.matmul(pb[:], zs[:, k, sl],
                                             di_sb[:, k, :],
                                             start=(k == 0), stop=(k == 3))
                    else:
                        for k in range(4):
                            nc.tensor.matmul(pa[:], zr[:, k, sl],
                                             dp_sb[:, k, :],
                                             start=(k == 0), stop=(k == 3))
                            nc.tensor.matmul(pb[:], zs[:, k, sl],
                                             di_sb[:, k, :],
                                             start=(k == 0), stop=(k == 3))
                    if direct_combine:
                        pa_rd = pa
                    else:
                        pa_rd = spool.tile([128, H], F32, tag="pas")
                        nc.scalar.copy(pa_rd[:], pa[:])
                    nc.vector.tensor_sub(o1r[:, m, :], pa_rd[:], pb[:])
                    pc = ps1bc.tile([128, H], F32, tag="c")
                    for k in range(4):
                        nc.tensor.matmul(pc[:], zd[:, k, sl], dr_sb[:, k, :],
                                         start=(k == 0), stop=(k == 3))
                    nc.vector.tensor_add(o1i[:, m, :], pa_rd[:], pc[:])
                if xs_big:
                    nc.vector.tensor_add(xs[:], o1r[:], o1i[:])
                else:
                    for m in range(4):
                        nc.vector.tensor_add(xs[:, m, :], o1r[:, m, :],
                                             o1i[:, m, :])

                def _chunked(p):
                    return chunk_acc is True or (chunk_acc == "last"
                                                 and p == PAIRS - 1)

                if not defer_s2:
                    stage2(o1r, o1i, xs, pr, _chunked(pr))
                elif pending is not None:
                    stage2(*pending, _chunked(pending[3]))
                if defer_s2:
                    pending = (o1r, o1i, xs, pr)
              if defer_s2:
                  stage2(*pending, _chunked(pending[3]))

              nc.sync.dma_start(out=out.ap(), in_=acc[:])

    nc.compile()
    return nc


BUILD = _build_nc_v2


def kernel(predictions, ground_truths, band_weights, band_masks):
    global last_results, last_nc, last_in_maps
    pred = np.ascontiguousarray(np.asarray(predictions, dtype=np.float32))
    gt = np.ascontiguousarray(np.asarray(ground_truths, dtype=np.float32))
    bw = np.asarray(band_weights, dtype=np.float64)
    bm = np.asarray(band_masks, dtype=np.float64)

    # host-side prep of tiny replicated constants
    wmap = np.einsum('b,bhw->hw', bw, bm)          # shifted coords
    wu = np.fft.ifftshift(wmap)                     # unshifted coords
    bf = ml_dtypes.bfloat16
    wtb = np.ascontiguousarray(wu.T.astype(bf))
    j = np.arange(H, dtype=np.float64)
    ang = 2.0 * np.pi * np.outer(j, j) / H
    scale = 1.0 / np.sqrt(H)
    drm = np.cos(ang) * scale
    dim = -np.sin(ang) * scale
    drb = np.ascontiguousarray(drm.astype(bf))
    dib = np.ascontiguousarray(dim.astype(bf))
    dpb = np.ascontiguousarray((drm + dim).astype(bf))
    dmb = np.ascontiguousarray((dim - drm).astype(bf))
    dnb = np.ascontiguousarray((-dim).astype(bf))

    pred_r = pred.reshape(N_CORES, IMGS_PER_CORE, H, H)
    gt_r = gt.reshape(N_CORES, IMGS_PER_CORE, H, H)
    in_maps = [
        {
            "pred": np.ascontiguousarray(pred_r[c]),
            "gt": np.ascontiguousarray(gt_r[c]),
            "d_r": drb, "d_i": dib, "d_p": dpb, "d_m": dmb, "d_n": dnb,
            "wt": wtb,
        }
        for c in range(N_CORES)
    ]

    nc = BUILD()
    last_nc, last_in_maps = nc, in_maps
    res = run_bass_kernel_spmd(nc, in_maps, core_ids=list(range(N_CORES)))
    last_results = res
    total = np.float64(0.0)
    for r in res.results:
        total += r["out"].astype(np.float64).sum()
    loss = total / float(N * C * H * H)
    return np.float32(loss)



# revision 37
# speedup vs baseline: 1.0594x; 1.0289x over previous
"""Trainium2 Bass kernel for DifferentiableWeightedRadialFrequencyLoss.

Math:
  loss = sum_{n,c,u,v} Wmap[u,v] * |FFT2(pred-gt)[u,v]|^2 / size
with Wmap = sum_b w_b * mask_b (bands disjoint), in unshifted (ifftshift)
frequency coordinates.

Device algorithm (per core, 12 images = 6 pairs):
  - pack two real images per complex FFT: Z = E1 + i*E2 (Wmap is symmetric
    under (u,v) -> (-u,-v), so cross terms cancel exactly).
  - FFT2 as two matmul stages with the symmetric ortho DFT matrix D:
      out1 = Z^T @ D      (stage 1)
      F^T  = D @ out1     (stage 2)
    each complex product via 3-multiplication Karatsuba:
      (A+iB)@(C+iD): m1=A@(C+D), m2=(A+B)@D, m3=(B-A)@C
                     real=m1-m2, imag=m1+m3
    with the constant-side combos (Dr+Di, Di-Dr) precomputed on host.
  - weighted power: P = Fr^2 + Fi^2 (ACT squares), acc += row-sum(P .* W^T)
    via DVE scalar_tensor_tensor accum_out.
Host: shard batch across 8 cores, sum partial accumulators, divide by size.
"""

import numpy as np
import ml_dtypes

import concourse.bass as bass
import concourse.bacc as bacc
import concourse.tile as tile
from concourse import mybir
from concourse.bass_utils import run_bass_kernel_spmd

N_CORES = 8
N, C, H = 32, 3, 512
NUM_BANDS = 16
IMGS_PER_CORE = (N // N_CORES) * C          # 12
PAIRS = IMGS_PER_CORE // 2                  # 6
F32 = mybir.dt.float32
BF16 = mybir.dt.bfloat16
ALU = mybir.AluOpType

# exposed for test.py introspection
last_results = None
last_nc = None
last_in_maps = None


def _build_nc(s2_3m=True, combo_eng="dve", xs_chunked=True,
              zbufs=3, o1bufs=2, ldbufs=2, tadd_eng="dve", spbufs=3,
              repeat=None):
    """repeat=N wraps the whole per-core body in a hardware For_i loop that
    re-executes it N times (identical work each iteration, including input
    and constant DMA).  Used by test.py to measure steady-state per-execution
    HW time by differencing two repeat counts; repeat=None is the normal
    single-shot kernel."""
    from contextlib import nullcontext
    nc = bacc.Bacc("TRN2", target_bir_lowering=False, debug=False,
                   num_devices=N_CORES)
    pred = nc.dram_tensor("pred", [IMGS_PER_CORE, H, H], F32, kind="ExternalInput")
    gt = nc.dram_tensor("gt", [IMGS_PER_CORE, H, H], F32, kind="ExternalInput")
    d_r = nc.dram_tensor("d_r", [H, H], BF16, kind="ExternalInput")   # Dr
    d_i = nc.dram_tensor("d_i", [H, H], BF16, kind="ExternalInput")   # Di
    d_p = nc.dram_tensor("d_p", [H, H], BF16, kind="ExternalInput")   # Dr+Di
    d_m = nc.dram_tensor("d_m", [H, H], BF16, kind="ExternalInput")   # Di-Dr
    d_n = nc.dram_tensor("d_n", [H, H], BF16, kind="ExternalInput")   # -Di
    wt = nc.dram_tensor("wt", [H, H], BF16, kind="ExternalInput")     # W^T
    out = nc.dram_tensor("out", [128, PAIRS], F32, kind="ExternalOutput")

    def r4(ap):  # [512, 512] dram view -> [128 part, 4 chunks, 512]
        return ap.rearrange("(c p) w -> p c w", p=128)

    with tile.TileContext(nc) as tc:
        with (
            tc.tile_pool(name="consts", bufs=1) as consts,
            tc.tile_pool(name="loads", bufs=ldbufs) as loads,
            tc.tile_pool(name="zpool", bufs=zbufs) as zpool,
            tc.tile_pool(name="o1pool", bufs=o1bufs) as o1pool,
            tc.tile_pool(name="spool", bufs=spbufs) as spool,
            tc.tile_pool(name="bigsc", bufs=2) as bigsc,
            tc.tile_pool(name="ps1a", bufs=2, space="PSUM") as ps1a,
            tc.tile_pool(name="ps1bc", bufs=1, space="PSUM") as ps1bc,
            tc.tile_pool(name="ps2a", bufs=2, space="PSUM") as ps2a,
            tc.tile_pool(name="ps2bc", bufs=1, space="PSUM") as ps2bc,
        ):
            dr_sb = consts.tile([128, 4, H], BF16)
            di_sb = consts.tile([128, 4, H], BF16)
            dp_sb = consts.tile([128, 4, H], BF16)
            dm_sb = consts.tile([128, 4, H], BF16)
            dn_sb = None if s2_3m else consts.tile([128, 4, H], BF16)
            wt_sb = consts.tile([128, 4, H], BF16)
            acc = consts.tile([128, PAIRS], F32)
            # warmup tile: keep PE busy during the initial DMA lead-in so
            # the HAM clock-gate is at full rate when real matmuls start.
            warm = consts.tile([128, H], BF16)
            nc.vector.memset(warm[:], 0.0)
            wps = ps2bc.tile([128, H], F32, tag="c2")
            NWARM = 16
            for i in range(NWARM):
                nc.tensor.matmul(wps[:], warm[:, 0:128], warm[:],
                                 start=(i == 0), stop=(i == NWARM - 1))

            rep_ctx = (
                tc.For_i(0, repeat, 1,
                         hint_engines=(mybir.EngineType.PE,
                                       mybir.EngineType.DVE))
                if repeat is not None else nullcontext()
            )
            with rep_ctx:
              for pr in range(PAIRS):
                if pr == 0:
                    pass
                i1, i2 = 2 * pr, 2 * pr + 1
                p1t = loads.tile([128, 4, H], F32, tag="p1t")
                g1t = loads.tile([128, 4, H], F32, tag="g1t")
                p2t = loads.tile([128, 4, H], F32, tag="p2t")
                g2t = loads.tile([128, 4, H], F32, tag="g2t")
                nc.sync.dma_start(out=p1t[:], in_=r4(pred.ap()[i1]))
                nc.sync.dma_start(out=g1t[:], in_=r4(gt.ap()[i1]))
                nc.sync.dma_start(out=p2t[:], in_=r4(pred.ap()[i2]))
                nc.sync.dma_start(out=g2t[:], in_=r4(gt.ap()[i2]))
                if pr == 0:
                    nc.sync.dma_start(out=dp_sb[:], in_=r4(d_p.ap()))
                    nc.sync.dma_start(out=di_sb[:], in_=r4(d_i.ap()))
                    nc.sync.dma_start(out=dr_sb[:], in_=r4(d_r.ap()))
                    nc.sync.dma_start(out=dm_sb[:], in_=r4(d_m.ap()))
                    if dn_sb is not None:
                        nc.sync.dma_start(out=dn_sb[:], in_=r4(d_n.ap()))
                    nc.sync.dma_start(out=wt_sb[:], in_=r4(wt.ap()))

                # data tiles: zr = E1, zi = E2, zs = zr+zi, zd = zi-zr
                zr = zpool.tile([128, 4, H], BF16, tag="zr")
                zi = zpool.tile([128, 4, H], BF16, tag="zi")
                zs = zpool.tile([128, 4, H], BF16, tag="zs")
                zd = zpool.tile([128, 4, H], BF16, tag="zd")
                ce = nc.gpsimd if combo_eng == "pool" else nc.vector
                nc.gpsimd.tensor_sub(zr[:], p1t[:], g1t[:])
                nc.vector.tensor_sub(zi[:], p2t[:], g2t[:])
                ce.tensor_add(zs[:], zr[:], zi[:])
                ce.tensor_sub(zd[:], zi[:], zr[:])

                # stage 1: out1 = Z^T @ D via 3M
                o1r = o1pool.tile([128, 4, H], BF16, tag="o1r")
                o1i = o1pool.tile([128, 4, H], BF16, tag="o1i")
                for m in range(4):
                    sl = slice(m * 128, (m + 1) * 128)
                    pa = ps1a.tile([128, H], F32, tag="a")
                    pb = ps1bc.tile([128, H], F32, tag="b")
                    for k in range(4):
                        nc.tensor.matmul(pa[:], zr[:, k, sl], dp_sb[:, k, :],
                                         start=(k == 0), stop=(k == 3))
                        nc.tensor.matmul(pb[:], zs[:, k, sl], di_sb[:, k, :],
                                         start=(k == 0), stop=(k == 3))
                    pa_sb = spool.tile([128, H], F32, tag="pas")
                    nc.scalar.copy(pa_sb[:], pa[:])
                    nc.vector.tensor_sub(o1r[:, m, :], pa_sb[:], pb[:])
                    pc = ps1bc.tile([128, H], F32, tag="c")
                    for k in range(4):
                        nc.tensor.matmul(pc[:], zd[:, k, sl], dr_sb[:, k, :],
                                         start=(k == 0), stop=(k == 3))
                    nc.vector.tensor_add(o1i[:, m, :], pa_sb[:], pc[:])
                xs = o1pool.tile([128, 4, H], BF16, tag="xs")
                if s2_3m:
                    if xs_chunked:
                        for m in range(4):
                            ce.tensor_add(xs[:, m, :], o1r[:, m, :], o1i[:, m, :])
                    else:
                        ce.tensor_add(xs[:], o1r[:], o1i[:])

                # stage 2: F^T = D @ out1 via 3M ; weighted power accumulate
                prt = bigsc.tile([128, 4, H], BF16, tag="prt")
                pit = bigsc.tile([128, 4, H], BF16, tag="pit")
                for v in range(4):
                    sl = slice(v * 128, (v + 1) * 128)
                    if s2_3m:
                        pa = ps2a.tile([128, H], F32, tag="a2")
                        pb = ps2bc.tile([128, H], F32, tag="b2")
                        for p in range(4):
                            nc.tensor.matmul(pa[:], dr_sb[:, p, sl], xs[:, p, :],
                                             start=(p == 0), stop=(p == 3))
                            nc.tensor.matmul(pb[:], dp_sb[:, p, sl], o1i[:, p, :],
                                             start=(p == 0), stop=(p == 3))
                        pa2_sb = spool.tile([128, H], F32, tag="pas2")
                        nc.scalar.copy(pa2_sb[:], pa[:])
                        fr = spool.tile([128, H], BF16, tag="fr")
                        nc.vector.tensor_sub(fr[:], pa2_sb[:], pb[:])
                        pc = ps2bc.tile([128, H], F32, tag="c2")
                        for p in range(4):
                            nc.tensor.matmul(pc[:], dm_sb[:, p, sl], o1r[:, p, :],
                                             start=(p == 0), stop=(p == 3))
                        fi = spool.tile([128, H], BF16, tag="fi")
                        nc.vector.tensor_add(fi[:], pa2_sb[:], pc[:])
                        nc.scalar.square(prt[:, v, :], fr[:])
                        nc.scalar.square(pit[:, v, :], fi[:])
                    else:
                        pa = ps2a.tile([128, H], F32, tag="a2")
                        pb = ps2a.tile([128, H], F32, tag="b2")
                        for p in range(4):
                            drp = dr_sb[:, p, sl]
                            nc.tensor.matmul(pa[:], drp, o1r[:, p, :],
                                             start=(p == 0), stop=False)
                            nc.tensor.matmul(pb[:], drp, o1i[:, p, :],
                                             start=(p == 0), stop=False)
                        for p in range(4):
                            nc.tensor.matmul(pa[:], dn_sb[:, p, sl], o1i[:, p, :],
                                             start=False, stop=(p == 3))
                            nc.tensor.matmul(pb[:], di_sb[:, p, sl], o1r[:, p, :],
                                             start=False, stop=(p == 3))
                        nc.scalar.square(prt[:, v, :], pa[:])
                        nc.scalar.square(pit[:, v, :], pb[:])
                t = bigsc.tile([128, 4, H], BF16, tag="t")
                te = nc.gpsimd if tadd_eng == "pool" else nc.vector
                te.tensor_add(t[:], prt[:], pit[:])
                gs = bigsc.tile([128, 4, H], BF16, tag="t")
                se = nc.gpsimd if tadd_eng == "pool" else nc.vector
                se.scalar_tensor_tensor(
                    out=gs[:], in0=t[:], scalar=0.0, in1=wt_sb[:],
                    op0=ALU.bypass, op1=ALU.mult,
                    accum_out=acc[:, pr: pr + 1])

            nc.sync.dma_start(out=out.ap(), in_=acc[:])

    nc.compile()
    return nc


def _build_nc_v2(repeat=None, zbufs=3, o1bufs=2, ldbufs=2, spbufs=3,
                 t_eng="dve", direct_combine=False, chunk_acc="last",
                 nwarm=16, defer_s2=True, xs_big=True, zi_eng="dve",
                 group_seq=True, stag_reset=False, head_chunk=True):
    """3M kernel, restructured vs _build_nc:
      - stage 2 of pair pr is emitted after stage 1 of pair pr+1 (defer_s2),
        so the PE instruction stream never stalls at the stage1->stage2
        dependency (stage2(pr)'s inputs are long ready by then)
      - the final pair's power-accumulate runs per v-chunk (chunk_acc="last")
        to shrink the kernel tail after the last matmul
      - optional variants kept as parameters for A/B: direct PSUM combines,
        engine choices, buffer depths."""
    from contextlib import nullcontext
    nc = bacc.Bacc("TRN2", target_bir_lowering=False, debug=False,
                   num_devices=N_CORES)
    pred = nc.dram_tensor("pred", [IMGS_PER_CORE, H, H], F32, kind="ExternalInput")
    gt = nc.dram_tensor("gt", [IMGS_PER_CORE, H, H], F32, kind="ExternalInput")
    d_r = nc.dram_tensor("d_r", [H, H], BF16, kind="ExternalInput")   # Dr
    d_i = nc.dram_tensor("d_i", [H, H], BF16, kind="ExternalInput")   # Di
    d_p = nc.dram_tensor("d_p", [H, H], BF16, kind="ExternalInput")   # Dr+Di
    d_m = nc.dram_tensor("d_m", [H, H], BF16, kind="ExternalInput")   # Di-Dr
    d_n = nc.dram_tensor("d_n", [H, H], BF16, kind="ExternalInput")   # -Di (unused)
    wt = nc.dram_tensor("wt", [H, H], BF16, kind="ExternalInput")     # W^T
    NACC = PAIRS * 4 if chunk_acc else PAIRS
    out = nc.dram_tensor("out", [128, NACC], F32, kind="ExternalOutput")
    # chunk_acc: True = per-v-chunk accumulate everywhere, "last" = only for
    # the final pair (shrinks the kernel tail without the per-op overhead
    # elsewhere), False = one big accumulate per pair.

    def r4(ap):
        return ap.rearrange("(c p) w -> p c w", p=128)

    with tile.TileContext(nc) as tc:
        with (
            tc.tile_pool(name="consts", bufs=1) as consts,
            tc.tile_pool(name="loads", bufs=ldbufs) as loads,
            tc.tile_pool(name="zpool", bufs=zbufs) as zpool,
            tc.tile_pool(name="o1pool", bufs=o1bufs) as o1pool,
            tc.tile_pool(name="spool", bufs=spbufs) as spool,
            tc.tile_pool(name="bigsc", bufs=2) as bigsc,
            tc.tile_pool(name="ps1a", bufs=2, space="PSUM") as ps1a,
            tc.tile_pool(name="ps1bc", bufs=1, space="PSUM") as ps1bc,
            tc.tile_pool(name="ps2a", bufs=2, space="PSUM") as ps2a,
            tc.tile_pool(name="ps2bc", bufs=1, space="PSUM") as ps2bc,
        ):
            dr_sb = consts.tile([128, 4, H], BF16)
            di_sb = consts.tile([128, 4, H], BF16)
            dp_sb = consts.tile([128, 4, H], BF16)
            dm_sb = consts.tile([128, 4, H], BF16)
            wt_sb = consts.tile([128, 4, H], BF16)
            acc = consts.tile([128, NACC], F32)
            warm = consts.tile([128, H], BF16)
            nc.vector.memset(warm[:], 0.0)
            wps = ps2bc.tile([128, H], F32, tag="c2")
            for i in range(nwarm):
                nc.tensor.matmul(wps[:], warm[:, 0:128], warm[:],
                                 start=(i == 0), stop=(i == nwarm - 1))

            if chunk_acc:
                nc.vector.memset(acc[:], 0.0)
            te = nc.gpsimd if t_eng == "pool" else nc.vector

            def stage2(o1r, o1i, xs, pr, chunked):
                # stage 2: F^T = D @ out1 via 3M
                prt = bigsc.tile([128, 4, H], BF16, tag="prt")
                pit = bigsc.tile([128, 4, H], BF16, tag="pit")
                for v in range(4):
                    sl = slice(v * 128, (v + 1) * 128)
                    pa = ps2a.tile([128, H], F32, tag="a2")
                    pb = ps2bc.tile([128, H], F32, tag="b2")
                    if group_seq:
                        for p in range(4):
                            nc.tensor.matmul(pa[:], dr_sb[:, p, sl],
                                             xs[:, p, :],
                                             start=(p == 0), stop=(p == 3))
                        for p in range(4):
                            nc.tensor.matmul(pb[:], dp_sb[:, p, sl],
                                             o1i[:, p, :],
                                             start=(p == 0), stop=(p == 3))
                    else:
                        for p in range(4):
                            nc.tensor.matmul(pa[:], dr_sb[:, p, sl],
                                             xs[:, p, :],
                                             start=(p == 0), stop=(p == 3))
                            nc.tensor.matmul(pb[:], dp_sb[:, p, sl],
                                             o1i[:, p, :],
                                             start=(p == 0), stop=(p == 3))
                    if direct_combine:
                        pa_rd = pa
                    else:
                        pa_rd = spool.tile([128, H], F32, tag="pas2")
                        nc.scalar.copy(pa_rd[:], pa[:])
                    fr = spool.tile([128, H], BF16, tag="fr")
                    nc.vector.tensor_sub(fr[:], pa_rd[:], pb[:])
                    pc = ps2bc.tile([128, H], F32, tag="c2")
                    for p in range(4):
                        nc.tensor.matmul(pc[:], dm_sb[:, p, sl], o1r[:, p, :],
                                         start=(p == 0), stop=(p == 3))
                    fi = spool.tile([128, H], BF16, tag="fi")
                    nc.vector.tensor_add(fi[:], pa_rd[:], pc[:])
                    nc.scalar.square(prt[:, v, :], fr[:])
                    nc.scalar.square(pit[:, v, :], fi[:])
                    if chunked:
                        tch = bigsc.tile([128, H], BF16, tag="t")
                        te.tensor_add(tch[:], prt[:, v, :], pit[:, v, :])
                        gch = bigsc.tile([128, H], BF16, tag="gs")
                        col = pr * 4 + v
                        nc.vector.scalar_tensor_tensor(
                            out=gch[:], in0=tch[:], scalar=0.0,
                            in1=wt_sb[:, v, :], op0=ALU.bypass, op1=ALU.mult,
                            accum_out=acc[:, col: col + 1])
                if not chunked:
                    t = bigsc.tile([128, 4, H], BF16, tag="t")
                    te.tensor_add(t[:], prt[:], pit[:])
                    gs = bigsc.tile([128, 4, H], BF16, tag="gs")
                    col = pr * 4 if chunk_acc else pr
                    nc.vector.scalar_tensor_tensor(
                        out=gs[:], in0=t[:], scalar=0.0, in1=wt_sb[:],
                        op0=ALU.bypass, op1=ALU.mult,
                        accum_out=acc[:, col: col + 1])

            rep_ctx = (
                tc.For_i(0, repeat, 1,
                         hint_engines=(mybir.EngineType.PE,
                                       mybir.EngineType.DVE),
                         staggered_reset=stag_reset)
                if repeat is not None else nullcontext()
            )
            with rep_ctx:
              pending = None
              for pr in range(PAIRS):
                i1, i2 = 2 * pr, 2 * pr + 1
                zie = nc.gpsimd if zi_eng == "pool" else nc.vector
                if head_chunk:
                    # Separate tiles per k-chunk: Tile deps are per-tile, so
                    # chunked tiles let the first matmuls start after chunk 0
                    # lands, not after the whole pair load (kills the
                    # post-barrier head stall).  Const DMAs are interleaved
                    # between chunks so dp/di/dr don't queue behind all 4MB
                    # of pair-0 loads.
                    zrk, zik, zsk, zdk = [], [], [], []
                    for c in range(4):
                        p1c = loads.tile([128, H], F32, tag=f"p1t{c}")
                        g1c = loads.tile([128, H], F32, tag=f"g1t{c}")
                        p2c = loads.tile([128, H], F32, tag=f"p2t{c}")
                        g2c = loads.tile([128, H], F32, tag=f"g2t{c}")
                        nc.sync.dma_start(out=p1c[:],
                                          in_=r4(pred.ap()[i1])[:, c, :])
                        nc.sync.dma_start(out=g1c[:],
                                          in_=r4(gt.ap()[i1])[:, c, :])
                        nc.sync.dma_start(out=p2c[:],
                                          in_=r4(pred.ap()[i2])[:, c, :])
                        nc.sync.dma_start(out=g2c[:],
                                          in_=r4(gt.ap()[i2])[:, c, :])
                        if pr == 0:
                            if c == 0:
                                nc.sync.dma_start(out=dp_sb[:],
                                                  in_=r4(d_p.ap()))
                            elif c == 1:
                                nc.sync.dma_start(out=di_sb[:],
                                                  in_=r4(d_i.ap()))
                            elif c == 2:
                                nc.sync.dma_start(out=dr_sb[:],
                                                  in_=r4(d_r.ap()))
                            else:
                                nc.sync.dma_start(out=dm_sb[:],
                                                  in_=r4(d_m.ap()))
                                nc.sync.dma_start(out=wt_sb[:],
                                                  in_=r4(wt.ap()))
                        zrc = zpool.tile([128, H], BF16, tag=f"zr{c}")
                        zic = zpool.tile([128, H], BF16, tag=f"zi{c}")
                        zsc = zpool.tile([128, H], BF16, tag=f"zs{c}")
                        zdc = zpool.tile([128, H], BF16, tag=f"zd{c}")
                        nc.gpsimd.tensor_sub(zrc[:], p1c[:], g1c[:])
                        zie.tensor_sub(zic[:], p2c[:], g2c[:])
                        nc.vector.tensor_add(zsc[:], zrc[:], zic[:])
                        nc.vector.tensor_sub(zdc[:], zic[:], zrc[:])
                        zrk.append(zrc); zik.append(zic)
                        zsk.append(zsc); zdk.append(zdc)
                else:
                    p1t = loads.tile([128, 4, H], F32, tag="p1t")
                    g1t = loads.tile([128, 4, H], F32, tag="g1t")
                    p2t = loads.tile([128, 4, H], F32, tag="p2t")
                    g2t = loads.tile([128, 4, H], F32, tag="g2t")
                    nc.sync.dma_start(out=p1t[:], in_=r4(pred.ap()[i1]))
                    nc.sync.dma_start(out=g1t[:], in_=r4(gt.ap()[i1]))
                    nc.sync.dma_start(out=p2t[:], in_=r4(pred.ap()[i2]))
                    nc.sync.dma_start(out=g2t[:], in_=r4(gt.ap()[i2]))
                    zr = zpool.tile([128, 4, H], BF16, tag="zr")
                    zi = zpool.tile([128, 4, H], BF16, tag="zi")
                    zs = zpool.tile([128, 4, H], BF16, tag="zs")
                    zd = zpool.tile([128, 4, H], BF16, tag="zd")
                    nc.gpsimd.tensor_sub(zr[:], p1t[:], g1t[:])
                    zie.tensor_sub(zi[:], p2t[:], g2t[:])
                    nc.vector.tensor_add(zs[:], zr[:], zi[:])
                    nc.vector.tensor_sub(zd[:], zi[:], zr[:])
                    zrk = [zr[:, c, :] for c in range(4)]
                    zsk = [zs[:, c, :] for c in range(4)]
                    zdk = [zd[:, c, :] for c in range(4)]
                    if pr == 0:
                        nc.sync.dma_start(out=dp_sb[:], in_=r4(d_p.ap()))
                        nc.sync.dma_start(out=di_sb[:], in_=r4(d_i.ap()))
                        nc.sync.dma_start(out=dr_sb[:], in_=r4(d_r.ap()))
                        nc.sync.dma_start(out=dm_sb[:], in_=r4(d_m.ap()))
                        nc.sync.dma_start(out=wt_sb[:], in_=r4(wt.ap()))

                # stage 1: out1 = Z^T @ D via 3M; combines read PSUM directly
                o1r = o1pool.tile([128, 4, H], BF16, tag="o1r")
                o1i = o1pool.tile([128, 4, H], BF16, tag="o1i")
                xs = o1pool.tile([128, 4, H], BF16, tag="xs")
                for m in range(4):
                    sl = slice(m * 128, (m + 1) * 128)
                    pa = ps1a.tile([128, H], F32, tag="a")
                    pb = ps1bc.tile([128, H], F32, tag="b")
                    if group_seq:
                        for k in range(4):
                            nc.tensor.matmul(pa[:], zrk[k][:, sl],
                                             dp_sb[:, k, :],
                                             start=(k == 0), stop=(k == 3))
                        for k in range(4):
                            nc.tensor.matmul(pb[:], zsk[k][:, sl],
                                             di_sb[:, k, :],
                                             start=(k == 0), stop=(k == 3))
                    else:
                        for k in range(4):
                            nc.tensor.matmul(pa[:], zrk[k][:, sl],
                                             dp_sb[:, k, :],
                                             start=(k == 0), stop=(k == 3))
                            nc.tensor.matmul(pb[:], zsk[k][:, sl],
                                             di_sb[:, k, :],
                                             start=(k == 0), stop=(k == 3))
                    if direct_combine:
                        pa_rd = pa
                    else:
                        pa_rd = spool.tile([128, H], F32, tag="pas")
                        nc.scalar.copy(pa_rd[:], pa[:])
                    nc.vector.tensor_sub(o1r[:, m, :], pa_rd[:], pb[:])
                    pc = ps1bc.tile([128, H], F32, tag="c")
                    for k in range(4):
                        nc.tensor.matmul(pc[:], zdk[k][:, sl],
                                         dr_sb[:, k, :],
                                         start=(k == 0), stop=(k == 3))
                    nc.vector.tensor_add(o1i[:, m, :], pa_rd[:], pc[:])
                if xs_big:
                    nc.vector.tensor_add(xs[:], o1r[:], o1i[:])
                else:
                    for m in range(4):
                        nc.vector.tensor_add(xs[:, m, :], o1r[:, m, :],
                                             o1i[:, m, :])

                def _chunked(p):
                    return chunk_acc is True or (chunk_acc == "last"
                                                 and p == PAIRS - 1)

                if not defer_s2:
                    stage2(o1r, o1i, xs, pr, _chunked(pr))
                elif pending is not None:
                    stage2(*pending, _chunked(pending[3]))
                if defer_s2:
                    pending = (o1r, o1i, xs, pr)
              if defer_s2:
                  stage2(*pending, _chunked(pending[3]))

              nc.sync.dma_start(out=out.ap(), in_=acc[:])

    nc.compile()
    return nc


BUILD = _build_nc_v2


def kernel(predictions, ground_truths, band_weights, band_masks):
    global last_results, last_nc, last_in_maps
    pred = np.ascontiguousarray(np.asarray(predictions, dtype=np.float32))
    gt = np.ascontiguousarray(np.asarray(ground_truths, dtype=np.float32))
    bw = np.asarray(band_weights, dtype=np.float64)
    bm = np.asarray(band_masks, dtype=np.float64)

    # host-side prep of tiny replicated constants
    wmap = np.einsum('b,bhw->hw', bw, bm)          # shifted coords
    wu = np.fft.ifftshift(wmap)                     # unshifted coords
    bf = ml_dtypes.bfloat16
    wtb = np.ascontiguousarray(wu.T.astype(bf))
    j = np.arange(H, dtype=np.float64)
    ang = 2.0 * np.pi * np.outer(j, j) / H
    scale = 1.0 / np.sqrt(H)
    drm = np.cos(ang) * scale
    dim = -np.sin(ang) * scale
    drb = np.ascontiguousarray(drm.astype(bf))
    dib = np.ascontiguousarray(dim.astype(bf))
    dpb = np.ascontiguousarray((drm + dim).astype(bf))
    dmb = np.ascontiguousarray((dim - drm).astype(bf))
    dnb = np.ascontiguousarray((-dim).astype(bf))

    pred_r = pred.reshape(N_CORES, IMGS_PER_CORE, H, H)
    gt_r = gt.reshape(N_CORES, IMGS_PER_CORE, H, H)
    in_maps = [
        {
            "pred": np.ascontiguousarray(pred_r[c]),
            "gt": np.ascontiguousarray(gt_r[c]),
            "d_r": drb, "d_i": dib, "d_p": dpb, "d_m": dmb, "d_n": dnb,
            "wt": wtb,
        }
        for c in range(N_CORES)
    ]

    nc = BUILD()
    last_nc, last_in_maps = nc, in_maps
    res = run_bass_kernel_spmd(nc, in_maps, core_ids=list(range(N_CORES)))
    last_results = res
    total = np.float64(0.0)
    for r in res.results:
        total += r["out"].astype(np.float64).sum()
    loss = total / float(N * C * H * H)
    return np.float32(loss)

